# revision 1
# baseline (speedup 1.0000x reference)
"""ChunkConformerBlock Trainium2 kernel.

Full inputs -> full output. Data-parallel over batch: B=16 sequences split
2-per-core across 8 NeuronCores; all parameters replicated. Each core runs
the whole conformer block (ff1 -> banded MHSA -> conv -> ff2 -> final LN)
on its 2048 tokens with activations kept in SBUF.

Layout strategy per core:
  - residual stream X: token-major fp32 tiles [128 tok, 512 d] (16 tiles)
  - LayerNorm: bn_stats/bn_aggr on DVE (token-major), fused scale+shift apply
  - matmul internals: feature-major bf16 (xn^T via PE transposes); weights
    pre-transposed host-side into [128, kchunks, N] bf16 SBUF layouts with
    LN gammas/betas, BN affine and 1/sqrt(dh) folded in
  - banded attention (band [i-6, i+2]): 136-wide key windows from guarded
    feature-major k^T; softmax token-major; attn transposed on PE for the
    attn@V matmul; shifted token-major v tiles so all matmul operands start
    at partition 0
  - depthwise causal conv K=32: 32 shifted fused multiply-adds
    (scalar_tensor_tensor) on DVE/GPSIMD over feature-major bf16 with
    zero guard columns per sequence
"""

import numpy as np
import ml_dtypes

import concourse.bass as bass
import concourse.tile as tile
from concourse import mybir
from concourse.bass_utils import run_bass_kernel_spmd

import json as _json

# ---------------------------------------------------------------------------
# walrus in this container supports at most ONE sync-wait command per
# instruction; Tile can emit more. Split extras onto standalone
# EventSemaphore instructions at the BIR-JSON level.
_wsplit_ctr = [0]


def _split_waits(bir_json: bytes, cap: int = 1) -> bytes:
    j = _json.loads(bir_json)
    changed = False
    for f in j.get("functions", []):
        for b in f.get("blocks", []):
            new_list = []
            for ins in b.get("instructions", []):
                si = ins.get("sync_info") or {}
                waits = si.get("on_wait") or []
                if len(waits) > cap:
                    extra, keep = waits[:-cap], waits[-cap:]
                    si["on_wait"] = keep
                    ins["sync_info"] = si
                    for w in extra:
                        _wsplit_ctr[0] += 1
                        new_list.append({
                            "engine": ins.get("engine"),
                            "ins": [], "outs": [],
                            "name": f"I-wsplit-{_wsplit_ctr[0]}",
                            "opcode": "EventSemaphore",
                            "sync_info": {"on_update": [], "on_wait": [w]},
                        })
                    changed = True
                new_list.append(ins)
            b["instructions"] = new_list
    return _json.dumps(j).encode() if changed else bir_json


_hook_installed = [False]


def _install_hook():
    if _hook_installed[0]:
        return
    from concourse import bass_utils as _bu, bass2jax as _b2j
    orig = _bu.compile_bir_kernel

    def patched(bir_json, tmpdir, neff_name="file.neff"):
        return orig(_split_waits(bir_json), tmpdir, neff_name)

    _bu.compile_bir_kernel = patched
    _b2j.compile_bir_kernel = patched
    _hook_installed[0] = True


# ---------------------------------------------------------------------------
f32 = mybir.dt.float32
bf16 = mybir.dt.bfloat16
AF = mybir.ActivationFunctionType
ALU = mybir.AluOpType
AX = mybir.AxisListType
bfnp = ml_dtypes.bfloat16

NCORES = 8
B, L, D = 16, 1024, 512
BL = B // NCORES              # sequences per core
NTOK = BL * L                 # tokens per core
P = 128
NT = NTOK // P                # 16 token tiles
DC = D // P                   # 4 d-chunks
F = 4 * D                     # 2048 ff hidden
FCH = F // P                  # 16 f-chunks
H, DH = 4, 128
KC = 32                       # conv taps
WF, WB = 6, 2
WIN = 136                     # key window = 128 + WF + WB
FC = 0.5
EPS = 1e-3
GK = 8                        # kT/xnT guard columns each side
GC = 32                       # gluT guard columns (per sequence, front)
T4 = 512                      # wide token tile for N=512 matmuls
NT4 = NTOK // T4              # 4
SL = GC + L                   # per-seq glu row length
TPS = L // P                  # tiles per sequence (8)
DVE_TAPS = 32                 # dwconv taps on DVE (gpsimd lacks stt)


def _chunk_k(w, n_chunk):
    """[K, N] host weight -> [128, n_chunk, N] SBUF layout (K on partitions)."""
    K, N = w.shape
    assert K == n_chunk * P
    return np.ascontiguousarray(w.reshape(n_chunk, P, N).transpose(1, 0, 2))


def _bias_cols(b, n_chunk):
    """[N] bias -> [128, n_chunk] per-partition bias columns."""
    assert b.shape[0] == n_chunk * P
    return np.ascontiguousarray(b.reshape(n_chunk, P).T)


def _chunk_mask_np(n):
    i = np.arange(n)[:, None]
    j = np.arange(n)[None, :]
    low = np.maximum(i - WF, 0)
    high = np.clip(i + WB, 0, n)
    low = low - np.maximum(low - n + WB, 0)
    high = np.maximum(high, WB)
    return (j >= low) & (j <= high)


def host_prep(inputs):
    """Fold LN gammas/betas, BN affine, 1/sqrt(dh); build SBUF-layout arrays."""
    g = {k: np.asarray(v, np.float32) for k, v in inputs.items()}
    out = {}

    for pfx, tag in (("ff1", "a"), ("ff2", "b")):
        w1 = g[pfx + "_g"][:, None] * g[pfx + "_w1"]
        b1 = g[pfx + "_b"] @ g[pfx + "_w1"] + g[pfx + "_b1"]
        out["w1" + tag] = _chunk_k(w1, DC).astype(bfnp)
        out["b1c" + tag] = _bias_cols(b1, FCH)
        out["w2" + tag] = _chunk_k(g[pfx + "_w2"], FCH).astype(bfnp)
        out["b2r" + tag] = g[pfx + "_b2"][None, :].astype(bfnp)

    sc = 1.0 / np.sqrt(DH)
    for nm, scale in (("wq", sc), ("wk", 1.0), ("wv", 1.0)):
        wf = g[nm].reshape(D, H * DH)
        bf_ = g["b" + nm[1]].reshape(H * DH)
        wp = (g["mh_g"][:, None] * wf) * scale
        bp = (g["mh_b"] @ wf + bf_) * scale
        out[nm + "s"] = _chunk_k(wp, DC).astype(bfnp)
        if nm != "wv":
            out["b" + nm[1] + "c"] = _bias_cols(bp, H)
        else:
            out["bvr"] = bp[None, :].astype(bfnp)
    out["wos"] = _chunk_k(g["wo"].reshape(H * DH, D), H).astype(bfnp)
    out["bor"] = g["bo"][None, :].astype(bfnp)

    pw1 = g["cv_g"][:, None] * g["pw1_w"]
    pw1b = g["cv_b"] @ g["pw1_w"] + g["pw1_b"]
    out["pw1s"] = _chunk_k(pw1, DC).astype(bfnp)
    out["pw1bc"] = _bias_cols(pw1b, 2 * DC)
    s = g["bn_g"] / np.sqrt(g["bn_v"] + EPS)
    t = g["bn_b"] - g["bn_m"] * s
    sepw = g["sep_w"] * s[None, :]
    sepb = g["sep_b"] * s + t
    out["seps"] = _chunk_k(sepw, DC).astype(bfnp)
    out["sepbc"] = _bias_cols(sepb, 2 * DC)
    out["pw2s"] = _chunk_k(g["pw2_w"], 2 * DC).astype(bfnp)
    out["pw2br"] = g["pw2_b"][None, :].astype(bfnp)

    out["dww"] = np.ascontiguousarray(
        g["dw_w"].T.reshape(DC, P, KC).transpose(1, 0, 2)).astype(np.float32)

    out["lngr"] = np.broadcast_to(g["ln_g"][None, :], (P, D)).astype(np.float32).copy()
    out["lnbr"] = np.broadcast_to(g["ln_b"][None, :], (P, D)).astype(np.float32).copy()

    m_full = _chunk_mask_np(L)
    masks = np.full((P, 3, WIN), -1e9, np.float32)
    for mi, t in ((0, 0), (1, 3), (2, TPS - 1)):
        i0 = t * P
        for r in range(P):
            for c in range(WIN):
                jj = i0 - WF + c
                if 0 <= jj < L and m_full[i0 + r, jj]:
                    masks[r, mi, c] = 0.0
    out["masks"] = masks
    out["masks2"] = np.repeat(masks[:, :, None, :], 2, axis=2).astype(bfnp)

    out["ident"] = np.eye(P, dtype=np.float32).astype(bfnp)
    out["onesr"] = np.ones((1, P), np.float32).astype(bfnp)
    out["b2ra32"] = out["b2ra"].astype(np.float32)
    out["b2rb32"] = out["b2rb"].astype(np.float32)
    out["bor32"] = out["bor"].astype(np.float32)
    out["pw2br32"] = out["pw2br"].astype(np.float32)
    out["bvr32"] = out["bvr"].astype(np.float32)
    return out


SPECS = {
    "w1a": ([P, DC, F], bf16), "b1ca": ([P, FCH], f32),
    "w2a": ([P, FCH, D], bf16), "b2ra": ([1, D], bf16),
    "w1b": ([P, DC, F], bf16), "b1cb": ([P, FCH], f32),
    "w2b": ([P, FCH, D], bf16), "b2rb": ([1, D], bf16),
    "wqs": ([P, DC, H * DH], bf16), "bqc": ([P, H], f32),
    "wks": ([P, DC, H * DH], bf16), "bkc": ([P, H], f32),
    "wvs": ([P, DC, H * DH], bf16), "bvr": ([1, H * DH], bf16),
    "wos": ([P, H, D], bf16), "bor": ([1, D], bf16),
    "pw1s": ([P, DC, 2 * D], bf16), "pw1bc": ([P, 2 * DC], f32),
    "seps": ([P, DC, 2 * D], bf16), "sepbc": ([P, 2 * DC], f32),
    "pw2s": ([P, 2 * DC, D], bf16), "pw2br": ([1, D], bf16),
    "dww": ([P, DC, KC], f32),
    "lngr": ([P, D], f32), "lnbr": ([P, D], f32),
    "masks": ([P, 3, WIN], f32),
    "masks2": ([P, 3, 2, WIN], bf16),
    "ident": ([P, P], bf16), "onesr": ([1, P], bf16),
    "b2ra32": ([1, D], f32), "b2rb32": ([1, D], f32),
    "bor32": ([1, D], f32), "pw2br32": ([1, D], f32),
    "bvr32": ([1, H * DH], f32),
}


# ---------------------------------------------------------------------------
def build_nc(debug_stages=False, trivial_final_gb=True, sim_safe=False, stages=15, zero_bias=False):
    nc = bass.Bass()

    x_d = nc.dram_tensor("x", [NTOK, D], f32, kind="ExternalInput")
    y_d = nc.dram_tensor("y", [NTOK, D], f32, kind="ExternalOutput")
    dbg = []
    if debug_stages:
        for i in range(4):
            dbg.append(nc.dram_tensor(f"dbg{i}", [NTOK, D], f32,
                                      kind="ExternalOutput"))

    wd = {}
    for nm, (shp, dt) in SPECS.items():
        wd[nm] = nc.dram_tensor(nm, shp, dt, kind="ExternalInput")

    with tile.TileContext(nc) as tc:
        _emit(nc, tc, x_d, y_d, wd, dbg, trivial_final_gb, sim_safe, stages, zero_bias)
    return nc


def _emit(nc, tc, x_d, y_d, wd, dbg, trivial_final_gb, sim_safe, stages=15, zero_bias=False):
    from contextlib import ExitStack
    ctx = ExitStack()
    with ctx:
        cpool = ctx.enter_context(tc.tile_pool(name="const", bufs=1))
        wpool = ctx.enter_context(tc.tile_pool(name="wts", bufs=1))
        xpool = ctx.enter_context(tc.tile_pool(name="xres", bufs=1))
        apool = ctx.enter_context(tc.tile_pool(name="acts", bufs=1))
        bpool = ctx.enter_context(tc.tile_pool(name="big", bufs=3))
        spool = ctx.enter_context(tc.tile_pool(name="small", bufs=2))
        stpool = ctx.enter_context(tc.tile_pool(name="stats", bufs=2))
        ppool = ctx.enter_context(tc.tile_pool(name="ps", bufs=3, space="PSUM"))
        pspool = ctx.enter_context(tc.tile_pool(name="pss", bufs=1, space="PSUM"))

        # ---- persistent consts -------------------------------------------
        cs = {}
        cnames = ["b1ca", "b2ra", "b1cb", "b2rb", "bqc", "bkc", "bvr",
                  "bor", "pw1bc", "sepbc", "pw2br", "dww",
                  "masks", "ident", "onesr"]
        if not trivial_final_gb:
            cnames += ["lngr", "lnbr"]
        for nm in cnames:
            shp, dt = SPECS[nm]
            t = cpool.tile(shp, dt, tag=nm, name=nm)
            eng = nc.sync if nm in ("ident", "dww") else nc.gpsimd
            eng.dma_start(out=t[:], in_=wd[nm][:])
            cs[nm] = t
        eps_t = cpool.tile([P, 1], f32, tag="eps", name="eps_t")
        nc.vector.memset(eps_t[:], EPS)

        # ---- X residual stream -------------------------------------------
        X = xpool.tile([P, NT, D], f32, tag="X", name="X")
        xr = x_d.rearrange("(t p) d -> p t d", p=P)
        for t in range(NT):
            nc.sync.dma_start(out=X[:, t, :], in_=xr[:, t, :])

        def load_w(nm, tag):
            shp, dt = SPECS[nm]
            t = wpool.tile(shp, dt, tag=tag, name=nm + "_sb")
            if len(shp) == 3 and shp[1] > 1:
                for c in range(shp[1]):
                    nc.gpsimd.dma_start(out=t[:, c, :], in_=wd[nm][:, c, :])
            else:
                nc.gpsimd.dma_start(out=t[:], in_=wd[nm][:])
            return t

        # ---- helpers ------------------------------------------------------
        def bias_accum(ps_ap, nm, width, rows=P):
            nc.gpsimd.dma_start(
                out=ps_ap, in_=wd[nm][:].to_broadcast([rows, width]),
                accum_op=ALU.add)

        def emit_silu(out_ap, psum_ap, bias_ap):
            if not sim_safe:
                nc.scalar.activation(out=out_ap, in_=psum_ap, func=AF.Silu,
                                     bias=bias_ap)
            else:
                sgt = spool.tile(list(out_ap.shape), bf16, tag="sgt", bufs=2,
                                 name="sgt")
                nc.scalar.activation(out=sgt[:], in_=psum_ap, func=AF.Sigmoid,
                                     bias=bias_ap)
                nc.vector.scalar_tensor_tensor(
                    out=out_ap, in0=psum_ap, scalar=bias_ap, in1=sgt[:],
                    op0=ALU.add, op1=ALU.mult)

        def emit_ln():
            rstds, nmrs = [], []
            for g in range(NT // 4):
                mv = stpool.tile([P, 4, 2], f32, tag="mv", bufs=3, name="mv")
                for j in range(4):
                    st6 = stpool.tile([P, 6], f32, tag="st6", bufs=3,
                                      name="st6")
                    nc.vector.bn_stats(out=st6[:], in_=X[:, g * 4 + j, :])
                    nc.vector.bn_aggr(out=mv[:, j, :], in_=st6[:])
                rstd = stpool.tile([P, 4], f32, tag="rstd", bufs=3,
                                   name="rstd")
                nmr = stpool.tile([P, 4], f32, tag="nmr", bufs=3, name="nmr")
                nc.scalar.activation(out=rstd[:], in_=mv[:, :, 1],
                                     func=AF.Sqrt, bias=eps_t[:])
                nc.vector.reciprocal(out=rstd[:], in_=rstd[:])
                nc.vector.scalar_tensor_tensor(
                    out=nmr[:], in0=mv[:, :, 0], scalar=-1.0, in1=rstd[:],
                    op0=ALU.mult, op1=ALU.mult)
                rstds.append(rstd)
                nmrs.append(nmr)
            return rstds, nmrs

        def emit_xnT(rstds, nmrs, guard=False):
            xnT = apool.tile([P, DC, NTOK + 2 * GK], bf16, tag="xnT",
                             name="xnT")
            if guard:
                for dcc in range(DC):
                    nc.gpsimd.memset(xnT[:, dcc, 0:GK], 0.0)
                    nc.gpsimd.memset(xnT[:, dcc, GK + NTOK:], 0.0)
            for t in range(NT):
                xn = spool.tile([P, D], bf16, tag="xn", bufs=3, name="xn")
                nc.vector.tensor_scalar(
                    out=xn[:], in0=X[:, t, :],
                    scalar1=rstds[t // 4][:, t % 4:t % 4 + 1],
                    scalar2=nmrs[t // 4][:, t % 4:t % 4 + 1],
                    op0=ALU.mult, op1=ALU.add)
                pt = pspool.tile([P, D], bf16, tag="ptr", bufs=1, name="pt")
                for dcc in range(DC):
                    nc.tensor.transpose(pt[:, dcc * P:(dcc + 1) * P],
                                        xn[:, dcc * P:(dcc + 1) * P],
                                        cs["ident"][:])
                nc.scalar.activation(
                    out=xnT[:, :, GK + t * P: GK + (t + 1) * P],
                    in_=pt[:].rearrange("p (c t) -> p c t", c=DC),
                    func=AF.Copy)
            return xnT

        def ff_module(w1, b1c, w2, b2r):
            rstds, nmrs = emit_ln()
            xnT = emit_xnT(rstds, nmrs)
            for t4 in range(NT4):
                tok0 = t4 * T4
                h1T = bpool.tile([P, FCH, T4], bf16, tag="big", name="h1T")
                for fc in range(FCH):
                    ps = ppool.tile([P, T4], f32, tag="pbig", name="psf")
                    for dcc in range(DC):
                        nc.tensor.matmul(
                            ps[:], w1[:, dcc, fc * P:(fc + 1) * P],
                            xnT[:, dcc, GK + tok0: GK + tok0 + T4],
                            start=(dcc == 0), stop=(dcc == DC - 1))
                    emit_silu(h1T[:, fc, :], ps[:], b1c[:, fc:fc + 1])
                for j in range(T4 // P):
                    t = (tok0 // P) + j
                    ps2 = ppool.tile([P, D], f32, tag="pbig", name="psb")
                    for fc in range(FCH):
                        nc.tensor.matmul(
                            ps2[:], h1T[:, fc, j * P:(j + 1) * P], w2[:, fc, :],
                            start=(fc == 0),
                            stop=(zero_bias and fc == FCH - 1))
                    if not zero_bias:
                        nc.tensor.matmul(ps2[:], cs["onesr"][:], b2r[:],
                                         start=False, stop=True)
                    nc.vector.scalar_tensor_tensor(
                        out=X[:, t, :], in0=ps2[:], scalar=FC, in1=X[:, t, :],
                        op0=ALU.mult, op1=ALU.add)

        def mhsa_module():
            rstds, nmrs = emit_ln()
            xnT = emit_xnT(rstds, nmrs, guard=True)
            wq = load_w("wqs", "wq")
            wk = load_w("wks", "wk")
            wv = load_w("wvs", "wv")
            wo = load_w("wos", "wo")

            qT = bpool.tile([P, H, NTOK], bf16, tag="big", name="qT")
            kT = bpool.tile([P, H, NTOK + 2 * GK], bf16, tag="big", name="kT")
            for h in range(H):
                nc.gpsimd.memset(kT[:, h, 0:GK], 0.0)
                nc.gpsimd.memset(kT[:, h, GK + NTOK:], 0.0)
            for h in range(H):
                for t4 in range(NT4):
                    tok0 = t4 * T4
                    psq = ppool.tile([P, T4], f32, tag="pbig", name="psq")
                    for dcc in range(DC):
                        nc.tensor.matmul(
                            psq[:], wq[:, dcc, h * DH:(h + 1) * DH],
                            xnT[:, dcc, GK + tok0: GK + tok0 + T4],
                            start=(dcc == 0), stop=(dcc == DC - 1))
                    nc.vector.tensor_scalar(
                        out=qT[:, h, tok0:tok0 + T4], in0=psq[:],
                        scalar1=cs["bqc"][:, h:h + 1], scalar2=None,
                        op0=ALU.add)
                    psk = ppool.tile([P, T4], f32, tag="pbig", name="psk")
                    for dcc in range(DC):
                        nc.tensor.matmul(
                            psk[:], wk[:, dcc, h * DH:(h + 1) * DH],
                            xnT[:, dcc, GK + tok0: GK + tok0 + T4],
                            start=(dcc == 0), stop=(dcc == DC - 1))
                    nc.vector.tensor_scalar(
                        out=kT[:, h, GK + tok0: GK + tok0 + T4], in0=psk[:],
                        scalar1=cs["bkc"][:, h:h + 1], scalar2=None,
                        op0=ALU.add)

            # shifted v tiles: tile st covers tokens [st*128-6, st*128+122)
            v_sb = bpool.tile([P, NT, H * DH], bf16, tag="big", name="v_sb")
            v17 = spool.tile([P, H * DH], bf16, tag="v17", bufs=1, name="v17")
            for st in range(NT + 1):
                m = P if st < NT else GK
                ps = ppool.tile([P, H * DH], f32, tag="pbig", name="psv")
                c0 = GK + st * P - WF
                for dcc in range(DC):
                    nc.tensor.matmul(
                        ps[:m, :], xnT[:, dcc, c0:c0 + m], wv[:, dcc, :],
                        start=(dcc == 0),
                        stop=(zero_bias and dcc == DC - 1))
                if not zero_bias:
                    nc.tensor.matmul(ps[:m, :], cs["onesr"][:, 0:m],
                                     cs["bvr"][:], start=False, stop=True)
                dst = v_sb[:, st, :] if st < NT else v17[0:GK, :]
                nc.vector.tensor_copy(dst, ps[:m, :])

            for st in range(NT):
                i0 = st * P
                tin = st % TPS
                mi = 0 if tin == 0 else (2 if tin == TPS - 1 else 1)
                oT_t = spool.tile([P, H, P], bf16, tag="oTt", bufs=2,
                                  name="oT_t")
                for hp in range(H // 2):
                    sps = pspool.tile([P, 2, WIN], f32, tag="psc", bufs=1,
                                      name="sps")
                    for hh in range(2):
                        h = hp * 2 + hh
                        nc.tensor.matmul(
                            sps[:, hh, :], qT[:, h, i0:i0 + P],
                            kT[:, h, GK + i0 - WF: GK + i0 - WF + WIN],
                            start=True, stop=True)
                    sm = spool.tile([P, 2, WIN], f32, tag="sm", name="sm")
                    for hh in range(2):
                        nc.vector.tensor_tensor(
                            out=sm[:, hh, :], in0=sps[:, hh, :],
                            in1=cs["masks"][:, mi, :], op=ALU.add)
                    ngm = spool.tile([P, 2], f32, tag="ngm", name="ngm")
                    nc.vector.tensor_reduce(out=ngm[:], in_=sm[:], axis=AX.X,
                                            op=ALU.max, negate=True)
                    for hh in range(2):
                        h = hp * 2 + hh
                        ex = spool.tile([P, WIN], bf16, tag="ex", name="ex")
                        den = spool.tile([P, 1], f32, tag="den", name="den")
                        nc.scalar.activation(
                            out=ex[:], in_=sm[:, hh, :], func=AF.Exp,
                            bias=ngm[:, hh:hh + 1], accum_out=den[:])
                        rden = spool.tile([P, 1], f32, tag="rden", name="rden")
                        nc.vector.reciprocal(out=rden[:], in_=den[:])
                        at = spool.tile([P, WIN], bf16, tag="at", name="at")
                        nc.vector.tensor_scalar_mul(at[:], ex[:], rden[:])
                        pt2 = pspool.tile([P, 2 * P], bf16, tag="pt2", bufs=2,
                                          name="pt2")
                        nc.tensor.transpose(pt2[:, 0:P], at[:, 0:P],
                                            cs["ident"][:])
                        nc.tensor.transpose(pt2[0:GK, P:P + P], at[:, P:WIN],
                                            cs["ident"][:])
                        aT = spool.tile([P, P], bf16, tag="aT", name="aT")
                        bT = spool.tile([GK, P], bf16, tag="bT", name="bT")
                        nc.scalar.activation(out=aT[:], in_=pt2[:, 0:P],
                                             func=AF.Copy)
                        nc.vector.tensor_copy(bT[:], pt2[0:GK, P:P + P])
                        po = pspool.tile([P, P], f32, tag="pav", bufs=1,
                                         name="po")
                        nc.tensor.matmul(po[:],
                                         v_sb[:, st, h * DH:(h + 1) * DH],
                                         aT[:], start=True, stop=False)
                        vn = (v_sb[0:GK, st + 1, h * DH:(h + 1) * DH]
                              if st + 1 < NT else v17[0:GK, :][:, h * DH:(h + 1) * DH])
                        nc.tensor.matmul(po[:, P - GK:P], vn,
                                         bT[:, P - GK:P],
                                         start=False, stop=True)
                        nc.scalar.activation(out=oT_t[:, h, :], in_=po[:],
                                             func=AF.Copy)
                # output projection + residual for this tile
                pso = ppool.tile([P, D], f32, tag="pbig", name="pso")
                for h in range(H):
                    nc.tensor.matmul(pso[:], oT_t[:, h, :], wo[:, h, :],
                                     start=(h == 0),
                                     stop=(zero_bias and h == H - 1))
                if not zero_bias:
                    nc.tensor.matmul(pso[:], cs["onesr"][:], cs["bor"][:],
                                     start=False, stop=True)
                nc.vector.tensor_tensor(out=X[:, st, :], in0=pso[:],
                                        in1=X[:, st, :], op=ALU.add)

        def conv_module():
            rstds, nmrs = emit_ln()
            xnT = emit_xnT(rstds, nmrs)
            pw1 = load_w("pw1s", "pw1")
            gluT = bpool.tile([P, DC, 2, SL], bf16, tag="big", name="gluT")
            for dcc in range(DC):
                for s in range(2):
                    nc.gpsimd.memset(gluT[:, dcc, s, 0:GC], 0.0)
            for fc in range(DC):
                for t4 in range(NT4):
                    tok0 = t4 * T4
                    s, hf = t4 // 2, t4 % 2
                    psa = ppool.tile([P, T4], f32, tag="pbig", name="psa")
                    psg = ppool.tile([P, T4], f32, tag="pbig", name="psg")
                    for dcc in range(DC):
                        nc.tensor.matmul(
                            psa[:], pw1[:, dcc, fc * P:(fc + 1) * P],
                            xnT[:, dcc, GK + tok0: GK + tok0 + T4],
                            start=(dcc == 0), stop=(dcc == DC - 1))
                    for dcc in range(DC):
                        nc.tensor.matmul(
                            psg[:], pw1[:, dcc, D + fc * P: D + (fc + 1) * P],
                            xnT[:, dcc, GK + tok0: GK + tok0 + T4],
                            start=(dcc == 0), stop=(dcc == DC - 1))
                    sg = spool.tile([P, T4], bf16, tag="sg", name="sg")
                    nc.scalar.activation(out=sg[:], in_=psg[:], func=AF.Sigmoid,
                                         bias=cs["pw1bc"][:, DC + fc:DC + fc + 1])
                    nc.vector.scalar_tensor_tensor(
                        out=gluT[:, fc, s, GC + hf * T4: GC + (hf + 1) * T4],
                        in0=psa[:], scalar=cs["pw1bc"][:, fc:fc + 1], in1=sg[:],
                        op0=ALU.add, op1=ALU.mult)

            sep = load_w("seps", "sep")
            convT = bpool.tile([P, DC, 2, L], bf16, tag="big", name="convT")
            # depthwise conv as 32 PSUM-accumulated diagonal matmuls per
            # (chunk, 512-token tile); diag(w_k) built by scaling identity
            for dcc in range(DC):
                dg = apool.tile([P, KC, P], bf16, tag="dg", bufs=1, name="dg")
                for k in range(KC):
                    nc.vector.tensor_scalar_mul(dg[:, k, :], cs["ident"][:],
                                                cs["dww"][:, dcc, k:k + 1])
                for t4 in range(NT4):
                    s, hf = t4 // 2, t4 % 2
                    psc_ = ppool.tile([P, T4], f32, tag="pbig", name="psc_")
                    for k in range(KC):
                        nc.tensor.matmul(
                            psc_[:], dg[:, k, :],
                            gluT[:, dcc, s, 1 + hf * T4 + k:
                                 1 + hf * T4 + k + T4],
                            start=(k == 0), stop=(k == KC - 1))
                    nc.vector.tensor_copy(
                        convT[:, dcc, s, hf * T4:(hf + 1) * T4], psc_[:])

            pw2 = load_w("pw2s", "pw2")
            for t4 in range(NT4):
                tok0 = t4 * T4
                s, hf = t4 // 2, t4 % 2
                silT = bpool.tile([P, 2 * DC, T4], bf16, tag="big",
                                  name="silT")
                for fc in range(2 * DC):
                    ps = ppool.tile([P, T4], f32, tag="pbig", name="pss1")
                    for dcc in range(DC):
                        nc.tensor.matmul(
                            ps[:], sep[:, dcc, fc * P:(fc + 1) * P],
                            convT[:, dcc, s, hf * T4:(hf + 1) * T4],
                            start=(dcc == 0), stop=(dcc == DC - 1))
                    emit_silu(silT[:, fc, :], ps[:], cs["sepbc"][:, fc:fc + 1])
                for j in range(T4 // P):
                    t = (tok0 // P) + j
                    ps2 = ppool.tile([P, D], f32, tag="pbig", name="pss2")
                    for fc in range(2 * DC):
                        nc.tensor.matmul(
                            ps2[:], silT[:, fc, j * P:(j + 1) * P],
                            pw2[:, fc, :], start=(fc == 0),
                            stop=(zero_bias and fc == 2 * DC - 1))
                    if not zero_bias:
                        nc.tensor.matmul(ps2[:], cs["onesr"][:],
                                         cs["pw2br"][:],
                                         start=False, stop=True)
                    nc.vector.tensor_tensor(out=X[:, t, :], in0=ps2[:],
                                            in1=X[:, t, :], op=ALU.add)

        def dump_dbg(i):
            if dbg:
                for t in range(NT):
                    Xc = spool.tile([P, D], f32, tag="dbgc", bufs=2,
                                    name="Xc")
                    nc.vector.tensor_copy(Xc[:], X[:, t, :])
                    nc.scalar.dma_start(
                        out=dbg[i].rearrange("(t p) d -> p t d", p=P)[:, t, :],
                        in_=Xc[:])

        # ---- pipeline -----------------------------------------------------
        if stages & 1:
            w1a = load_w("w1a", "w1")
            w2a = load_w("w2a", "w2")
            ff_module(w1a, cs["b1ca"], w2a, cs["b2ra"])
            dump_dbg(0)
        if stages & 2:
            mhsa_module()
            dump_dbg(1)
        if stages & 4:
            conv_module()
            dump_dbg(2)
        if stages & 8:
            w1b = load_w("w1b", "w1")
            w2b = load_w("w2b", "w2")
            ff_module(w1b, cs["b1cb"], w2b, cs["b2rb"])
            dump_dbg(3)

        # ---- final LN + store --------------------------------------------
        rstds, nmrs = emit_ln()
        for t in range(NT):
            xo = spool.tile([P, D], f32, tag="xo", name="xo")
            nc.vector.tensor_scalar(
                out=xo[:], in0=X[:, t, :],
                scalar1=rstds[t // 4][:, t % 4:t % 4 + 1],
                scalar2=nmrs[t // 4][:, t % 4:t % 4 + 1],
                op0=ALU.mult, op1=ALU.add)
            if not trivial_final_gb:
                nc.vector.tensor_tensor(out=xo[:], in0=xo[:],
                                        in1=cs["lngr"][:], op=ALU.mult)
                nc.vector.tensor_tensor(out=xo[:], in0=xo[:],
                                        in1=cs["lnbr"][:], op=ALU.add)
            nc.scalar.dma_start(
                out=y_d.rearrange("(t p) d -> p t d", p=P)[:, t, :], in_=xo[:])


# ---------------------------------------------------------------------------
_cache = {}


def get_nc(debug_stages=False, trivial_final_gb=True, sim_safe=False,
           stages=15, zero_bias=False):
    key = ("nc", debug_stages, trivial_final_gb, sim_safe, stages, zero_bias)
    if key not in _cache:
        _install_hook()
        _cache[key] = build_nc(debug_stages, trivial_final_gb, sim_safe,
                               stages, zero_bias)
    return _cache[key]


def make_in_maps(inputs, debug_stages=False):
    prep = host_prep(inputs)
    x = np.asarray(inputs["inputs"], np.float32)
    trivial = (np.all(np.asarray(inputs["ln_g"]) == 1.0)
               and np.all(np.asarray(inputs["ln_b"]) == 0.0))
    zero_bias = all(
        not np.any(prep[k]) for k in ("b2ra", "b2rb", "bvr", "bor", "pw2br"))
    if not trivial:
        pass
    in_maps = []
    for c in range(NCORES):
        m = dict(prep)
        m["x"] = np.ascontiguousarray(x[c * BL:(c + 1) * BL].reshape(NTOK, D))
        in_maps.append(m)
    return in_maps, trivial, zero_bias


def kernel(**inputs):
    _install_hook()
    in_maps, trivial, zero_bias = make_in_maps(inputs)
    nc = get_nc(trivial_final_gb=trivial, zero_bias=zero_bias)
    res = run_bass_kernel_spmd(nc, in_maps, list(range(NCORES)))
    outs = [res.results[c]["y"].reshape(BL, L, D) for c in range(NCORES)]
    return np.concatenate(outs, axis=0)



# revision 20
# speedup vs baseline: 1.3576x; 1.3576x over previous
"""ChunkConformerBlock Trainium2 kernel.

Full inputs -> full output. Data-parallel over batch: B=16 sequences split
2-per-core across 8 NeuronCores; all parameters replicated. Each core runs
the whole conformer block (ff1 -> banded MHSA -> conv -> ff2 -> final LN)
on its 2048 tokens with activations kept in SBUF.

Layout strategy per core:
  - residual stream X: token-major fp32 tiles [128 tok, 512 d] (16 tiles)
  - LayerNorm: bn_stats/bn_aggr on DVE (token-major), fused scale+shift apply
  - matmul internals: feature-major bf16 (xn^T via PE transposes); weights
    pre-transposed host-side into [128, kchunks, N] bf16 SBUF layouts with
    LN gammas/betas, BN affine and 1/sqrt(dh) folded in
  - banded attention (band [i-6, i+2]): 136-wide key windows from guarded
    feature-major k^T; softmax token-major; attn transposed on PE for the
    attn@V matmul; shifted token-major v tiles so all matmul operands start
    at partition 0
  - depthwise causal conv K=32: 32 shifted fused multiply-adds
    (scalar_tensor_tensor) on DVE/GPSIMD over feature-major bf16 with
    zero guard columns per sequence
"""

import numpy as np
import ml_dtypes

import concourse.bass as bass
import concourse.tile as tile
from concourse import mybir
from concourse.bass_utils import run_bass_kernel_spmd

import json as _json

# ---------------------------------------------------------------------------
# walrus in this container supports at most ONE sync-wait command per
# instruction; Tile can emit more. Split extras onto standalone
# EventSemaphore instructions at the BIR-JSON level.
_wsplit_ctr = [0]


def _split_waits(bir_json: bytes, cap: int = 1) -> bytes:
    j = _json.loads(bir_json)
    changed = False
    for f in j.get("functions", []):
        for b in f.get("blocks", []):
            new_list = []
            for ins in b.get("instructions", []):
                si = ins.get("sync_info") or {}
                waits = si.get("on_wait") or []
                if len(waits) > cap:
                    extra, keep = waits[:-cap], waits[-cap:]
                    si["on_wait"] = keep
                    ins["sync_info"] = si
                    for w in extra:
                        _wsplit_ctr[0] += 1
                        new_list.append({
                            "engine": ins.get("engine"),
                            "ins": [], "outs": [],
                            "name": f"I-wsplit-{_wsplit_ctr[0]}",
                            "opcode": "EventSemaphore",
                            "sync_info": {"on_update": [], "on_wait": [w]},
                        })
                    changed = True
                new_list.append(ins)
            b["instructions"] = new_list
    return _json.dumps(j).encode() if changed else bir_json


_hook_installed = [False]


def _install_hook():
    if _hook_installed[0]:
        return
    from concourse import bass_utils as _bu, bass2jax as _b2j
    orig = _bu.compile_bir_kernel

    def patched(bir_json, tmpdir, neff_name="file.neff"):
        return orig(_split_waits(bir_json), tmpdir, neff_name)

    _bu.compile_bir_kernel = patched
    _b2j.compile_bir_kernel = patched
    _hook_installed[0] = True


# ---------------------------------------------------------------------------
f32 = mybir.dt.float32
bf16 = mybir.dt.bfloat16
f8 = mybir.dt.float8e4
AF = mybir.ActivationFunctionType
ALU = mybir.AluOpType
AX = mybir.AxisListType
DR = mybir.MatmulPerfMode.DoubleRow
bfnp = ml_dtypes.bfloat16
f8np = mybir.dt.np(mybir.dt.float8e4)
WS = 128.0                    # fp8 weight pre-scale (undone on psum read)
SCV = 64.0                    # dwconv output stored-scale (convT = 64*conv)
CS = SCV / (WS * WS)          # conv psum -> convT multiplier

NCORES = 8
B, L, D = 16, 1024, 512
BL = B // NCORES              # sequences per core
NTOK = BL * L                 # tokens per core
P = 128
NT = NTOK // P                # 16 token tiles
DC = D // P                   # 4 d-chunks
F = 4 * D                     # 2048 ff hidden
FCH = F // P                  # 16 f-chunks
H, DH = 4, 128
KC = 32                       # conv taps
WF, WB = 6, 2
WIN = 136                     # key window = 128 + WF + WB
FC = 0.5
EPS = 1e-3
GK = 8                        # kT/xnT guard columns each side
GC = 32                       # gluT guard columns (per sequence, front)
T4 = 512                      # wide token tile for N=512 matmuls
NT4 = NTOK // T4              # 4
SL = GC + L                   # per-seq glu row length
TPS = L // P                  # tiles per sequence (8)
DVE_TAPS = 32                 # dwconv taps on DVE (gpsimd lacks stt)


def _chunk_k(w, n_chunk):
    """[K, N] host weight -> [128, n_chunk, N] SBUF layout (K on partitions)."""
    K, N = w.shape
    assert K == n_chunk * P
    return np.ascontiguousarray(w.reshape(n_chunk, P, N).transpose(1, 0, 2))


def _bias_cols(b, n_chunk):
    """[N] bias -> [128, n_chunk] per-partition bias columns."""
    assert b.shape[0] == n_chunk * P
    return np.ascontiguousarray(b.reshape(n_chunk, P).T)


def _chunk_mask_np(n):
    i = np.arange(n)[:, None]
    j = np.arange(n)[None, :]
    low = np.maximum(i - WF, 0)
    high = np.clip(i + WB, 0, n)
    low = low - np.maximum(low - n + WB, 0)
    high = np.maximum(high, WB)
    return (j >= low) & (j <= high)


def host_prep(inputs):
    """Fold LN gammas/betas, BN affine, 1/sqrt(dh); build SBUF-layout arrays."""
    g = {k: np.asarray(v, np.float32) for k, v in inputs.items()}
    out = {}

    for pfx, tag in (("ff1", "a"), ("ff2", "b")):
        w1 = g[pfx + "_g"][:, None] * g[pfx + "_w1"]
        b1 = g[pfx + "_b"] @ g[pfx + "_w1"] + g[pfx + "_b1"]
        out["w1" + tag] = _chunk_k(w1 * WS, DC).astype(f8np)
        out["b1c" + tag] = _bias_cols(b1, FCH)
        out["w2" + tag] = _chunk_k(g[pfx + "_w2"] * WS, FCH).astype(f8np)
        out["b2r" + tag] = (g[pfx + "_b2"] * WS)[None, :].astype(bfnp)

    sc = 1.0 / np.sqrt(DH)
    for nm, scale in (("wq", sc), ("wk", 1.0), ("wv", 1.0)):
        wf = g[nm].reshape(D, H * DH)
        bf_ = g["b" + nm[1]].reshape(H * DH)
        wp = (g["mh_g"][:, None] * wf) * scale
        bp = (g["mh_b"] @ wf + bf_) * scale
        out[nm + "s"] = _chunk_k(wp, DC).astype(bfnp)
        if nm != "wv":
            out["b" + nm[1] + "c"] = _bias_cols(bp, H)
        else:
            out["bvr"] = bp[None, :].astype(bfnp)
    out["wos"] = _chunk_k(g["wo"].reshape(H * DH, D), H).astype(bfnp)
    out["bor"] = g["bo"][None, :].astype(bfnp)

    pw1 = g["cv_g"][:, None] * g["pw1_w"]
    pw1b = g["cv_b"] @ g["pw1_w"] + g["pw1_b"]
    out["pw1s"] = _chunk_k(pw1 * WS, DC).astype(f8np)
    # a-half bias pre-scaled by WS so gluT = WS * glu_true (one stt keeps
    # (psa + WS*b_a) * sigmoid); gate-half bias unscaled (applied after
    # scale=1/WS inside the Sigmoid activation).
    pw1b_sc = pw1b.copy()
    pw1b_sc[:D] *= WS
    out["pw1bc"] = _bias_cols(pw1b_sc, 2 * DC)
    s = g["bn_g"] / np.sqrt(g["bn_v"] + EPS)
    t = g["bn_b"] - g["bn_m"] * s
    sepw = g["sep_w"] * s[None, :]
    sepb = g["sep_b"] * s + t
    out["seps"] = _chunk_k(sepw * WS, DC).astype(f8np)
    out["sepbc"] = _bias_cols(sepb, 2 * DC)
    out["pw2s"] = _chunk_k(g["pw2_w"] * WS, 2 * DC).astype(f8np)
    out["pw2br"] = (g["pw2_b"] * WS)[None, :].astype(bfnp)

    out["dww"] = np.ascontiguousarray(
        g["dw_w"].T.reshape(DC, P, KC).transpose(1, 0, 2)).astype(np.float32)
    # paired diag taps for fp8 DoubleRow dwconv:
    # dgall[p, dcc, kp, i, m] = WS * dw_w[kp + 16*i, dcc*128+p] iff m == p
    dg = np.zeros((P, DC, KC // 2, 2, P), np.float32)
    for dcc in range(DC):
        for kp in range(KC // 2):
            for i in range(2):
                w = g["dw_w"][kp + 16 * i, dcc * P:(dcc + 1) * P] * WS
                dg[np.arange(P), dcc, kp, i, np.arange(P)] = w
    out["dgall"] = dg.astype(f8np)

    out["lngr"] = np.broadcast_to(g["ln_g"][None, :], (P, D)).astype(np.float32).copy()
    out["lnbr"] = np.broadcast_to(g["ln_b"][None, :], (P, D)).astype(np.float32).copy()

    m_full = _chunk_mask_np(L)
    masks = np.full((P, 3, WIN), -1e9, np.float32)
    for mi, t in ((0, 0), (1, 3), (2, TPS - 1)):
        i0 = t * P
        for r in range(P):
            for c in range(WIN):
                jj = i0 - WF + c
                if 0 <= jj < L and m_full[i0 + r, jj]:
                    masks[r, mi, c] = 0.0
    out["masks"] = masks
    out["masks2"] = np.repeat(masks[:, :, None, :], 2, axis=2).astype(bfnp)

    out["ident"] = np.eye(P, dtype=np.float32).astype(bfnp)
    out["onesr"] = np.ones((1, P), np.float32).astype(bfnp)
    out["b2ra32"] = out["b2ra"].astype(np.float32)
    out["b2rb32"] = out["b2rb"].astype(np.float32)
    out["bor32"] = out["bor"].astype(np.float32)
    out["pw2br32"] = out["pw2br"].astype(np.float32)
    out["bvr32"] = out["bvr"].astype(np.float32)
    return out


SPECS = {
    "w1a": ([P, DC, F], f8), "b1ca": ([P, FCH], f32),
    "w2a": ([P, FCH, D], f8), "b2ra": ([1, D], bf16),
    "w1b": ([P, DC, F], f8), "b1cb": ([P, FCH], f32),
    "w2b": ([P, FCH, D], f8), "b2rb": ([1, D], bf16),
    "wqs": ([P, DC, H * DH], bf16), "bqc": ([P, H], f32),
    "wks": ([P, DC, H * DH], bf16), "bkc": ([P, H], f32),
    "wvs": ([P, DC, H * DH], bf16), "bvr": ([1, H * DH], bf16),
    "wos": ([P, H, D], bf16), "bor": ([1, D], bf16),
    "pw1s": ([P, DC, 2 * D], f8), "pw1bc": ([P, 2 * DC], f32),
    "seps": ([P, DC, 2 * D], f8), "sepbc": ([P, 2 * DC], f32),
    "pw2s": ([P, 2 * DC, D], f8), "pw2br": ([1, D], bf16),
    "dww": ([P, DC, KC], f32),
    "dgall": ([P, DC, KC // 2, 2, P], f8),
    "lngr": ([P, D], f32), "lnbr": ([P, D], f32),
    "masks": ([P, 3, WIN], f32),
    "masks2": ([P, 3, 2, WIN], bf16),
    "ident": ([P, P], bf16), "onesr": ([1, P], bf16),
    "b2ra32": ([1, D], f32), "b2rb32": ([1, D], f32),
    "bor32": ([1, D], f32), "pw2br32": ([1, D], f32),
    "bvr32": ([1, H * DH], f32),
}


# ---------------------------------------------------------------------------
def build_nc(debug_stages=False, trivial_final_gb=True, sim_safe=False, stages=15, zero_bias=False):
    nc = bass.Bass()

    x_d = nc.dram_tensor("x", [NTOK, D], f32, kind="ExternalInput")
    y_d = nc.dram_tensor("y", [NTOK, D], f32, kind="ExternalOutput")
    dbg = []
    if debug_stages:
        for i in range(4):
            dbg.append(nc.dram_tensor(f"dbg{i}", [NTOK, D], f32,
                                      kind="ExternalOutput"))

    wd = {}
    for nm, (shp, dt) in SPECS.items():
        wd[nm] = nc.dram_tensor(nm, shp, dt, kind="ExternalInput")

    with tile.TileContext(nc) as tc:
        _emit(nc, tc, x_d, y_d, wd, dbg, trivial_final_gb, sim_safe, stages, zero_bias)
    return nc


def _emit(nc, tc, x_d, y_d, wd, dbg, trivial_final_gb, sim_safe, stages=15, zero_bias=False):
    from contextlib import ExitStack
    ctx = ExitStack()
    with ctx:
        cpool = ctx.enter_context(tc.tile_pool(name="const", bufs=1))
        wpool = ctx.enter_context(tc.tile_pool(name="wts", bufs=1))
        xpool = ctx.enter_context(tc.tile_pool(name="xres", bufs=1))
        apool = ctx.enter_context(tc.tile_pool(name="acts", bufs=1))
        bpool = ctx.enter_context(tc.tile_pool(name="big", bufs=3))
        spool = ctx.enter_context(tc.tile_pool(name="small", bufs=2))
        stpool = ctx.enter_context(tc.tile_pool(name="stats", bufs=2))
        ppool = ctx.enter_context(tc.tile_pool(name="ps", bufs=3, space="PSUM"))
        pspool = ctx.enter_context(tc.tile_pool(name="pss", bufs=1, space="PSUM"))

        # ---- X residual stream (split across queues; tiles 0-3 first) ----
        dmaengs = [nc.sync, nc.scalar, nc.vector, nc.gpsimd]
        X = xpool.tile([P, NT, D], f32, tag="X", name="X")
        xr = x_d.rearrange("(t p) d -> p t d", p=P)
        for t in range(NT):
            dmaengs[t % 4].dma_start(out=X[:, t, :], in_=xr[:, t, :])

        def load_w(nm, tag, engs=(nc.gpsimd,)):
            shp, dt = SPECS[nm]
            t = wpool.tile(shp, dt, tag=tag, name=nm + "_sb")
            if len(shp) == 3 and shp[1] > 1:
                for c in range(shp[1]):
                    engs[c % len(engs)].dma_start(
                        out=t[:, c, :], in_=wd[nm][:, c, :])
            else:
                engs[0].dma_start(out=t[:], in_=wd[nm][:])
            return t

        # ---- persistent consts -------------------------------------------
        cs = {}
        cnames = ["b1ca", "b2ra", "b1cb", "b2rb", "bqc", "bkc", "bvr",
                  "bor", "pw1bc", "sepbc", "pw2br",
                  "masks", "ident", "onesr"]
        if not trivial_final_gb:
            cnames += ["lngr", "lnbr"]
        for nm in cnames:
            shp, dt = SPECS[nm]
            t = cpool.tile(shp, dt, tag=nm, name=nm)
            eng = nc.sync if nm in ("ident",) else nc.scalar
            eng.dma_start(out=t[:], in_=wd[nm][:])
            cs[nm] = t
        eps_t = cpool.tile([P, 1], f32, tag="eps", name="eps_t")
        nc.vector.memset(eps_t[:], EPS)

        # ---- helpers ------------------------------------------------------
        def bias_accum(ps_ap, nm, width, rows=P):
            nc.gpsimd.dma_start(
                out=ps_ap, in_=wd[nm][:].to_broadcast([rows, width]),
                accum_op=ALU.add)

        def emit_silu(out_ap, psum_ap, bias_ap, scale=1.0):
            nc.scalar.activation(out=out_ap, in_=psum_ap, func=AF.Silu,
                                 bias=bias_ap, scale=scale)

        def emit_ln():
            rstds, nmrs = [], []
            for g in range(NT // 4):
                mv = stpool.tile([P, 4, 2], f32, tag="mv", bufs=3, name="mv")
                for j in range(4):
                    st6 = stpool.tile([P, 6], f32, tag="st6", bufs=3,
                                      name="st6")
                    nc.vector.bn_stats(out=st6[:], in_=X[:, g * 4 + j, :])
                    nc.vector.bn_aggr(out=mv[:, j, :], in_=st6[:])
                rstd = stpool.tile([P, 4], f32, tag="rstd", bufs=3,
                                   name="rstd")
                nmr = stpool.tile([P, 4], f32, tag="nmr", bufs=3, name="nmr")
                nc.scalar.activation(out=rstd[:], in_=mv[:, :, 1],
                                     func=AF.Sqrt, bias=eps_t[:])
                nc.vector.reciprocal(out=rstd[:], in_=rstd[:])
                nc.vector.scalar_tensor_tensor(
                    out=nmr[:], in0=mv[:, :, 0], scalar=-1.0, in1=rstd[:],
                    op0=ALU.mult, op1=ALU.mult)
                rstds.append(rstd)
                nmrs.append(nmr)
            return rstds, nmrs

        def emit_xnT(rstds, nmrs, guard=False, dt=bf16, tag="xnT"):
            xnT = apool.tile([P, DC, NTOK + 2 * GK], dt, tag=tag,
                             name=tag)
            if guard:
                for dcc in range(DC):
                    nc.gpsimd.memset(xnT[:, dcc, 0:GK], 0.0)
                    nc.gpsimd.memset(xnT[:, dcc, GK + NTOK:], 0.0)
            for t in range(NT):
                xn = spool.tile([P, D], bf16, tag="xn", bufs=3, name="xn")
                nc.vector.tensor_scalar(
                    out=xn[:], in0=X[:, t, :],
                    scalar1=rstds[t // 4][:, t % 4:t % 4 + 1],
                    scalar2=nmrs[t // 4][:, t % 4:t % 4 + 1],
                    op0=ALU.mult, op1=ALU.add)
                pt = pspool.tile([P, D], bf16, tag="ptr", bufs=1, name="pt")
                for dcc in range(DC):
                    nc.tensor.transpose(pt[:, dcc * P:(dcc + 1) * P],
                                        xn[:, dcc * P:(dcc + 1) * P],
                                        cs["ident"][:])
                nc.scalar.activation(
                    out=xnT[:, :, GK + t * P: GK + (t + 1) * P],
                    in_=pt[:].rearrange("p (c t) -> p c t", c=DC),
                    func=AF.Copy)
            return xnT

        def ff_module(w1, b1c, w2, b2r):
            rstds, nmrs = emit_ln()
            xnT = emit_xnT(rstds, nmrs, dt=f8, tag="xnT8")
            for t4 in range(NT4):
                tok0 = t4 * T4
                h1T = bpool.tile([P, FCH, T4], f8, tag="big", name="h1T")
                for fc in range(FCH):
                    ps = ppool.tile([P, T4], f32, tag="pbig", name="psf")
                    for c in range(DC // 2):
                        nc.tensor.matmul(
                            ps[:], w1[:, 2 * c:2 * c + 2, fc * P:(fc + 1) * P],
                            xnT[:, 2 * c:2 * c + 2, GK + tok0: GK + tok0 + T4],
                            start=(c == 0), stop=(c == DC // 2 - 1),
                            perf_mode=DR)
                    emit_silu(h1T[:, fc, :], ps[:], b1c[:, fc:fc + 1],
                              scale=1.0 / WS)
                for j in range(T4 // P):
                    t = (tok0 // P) + j
                    ps2 = ppool.tile([P, D], f32, tag="pbig", name="psb")
                    for c in range(FCH // 2):
                        nc.tensor.matmul(
                            ps2[:], h1T[:, 2 * c:2 * c + 2, j * P:(j + 1) * P],
                            w2[:, 2 * c:2 * c + 2, :],
                            start=(c == 0),
                            stop=(zero_bias and c == FCH // 2 - 1),
                            perf_mode=DR)
                    if not zero_bias:
                        # b2r host-scaled by WS so the FC/WS unscale is right
                        nc.tensor.matmul(ps2[:], cs["onesr"][:], b2r[:],
                                         start=False, stop=True)
                    nc.vector.scalar_tensor_tensor(
                        out=X[:, t, :], in0=ps2[:], scalar=FC / WS,
                        in1=X[:, t, :], op0=ALU.mult, op1=ALU.add)

        def mhsa_module():
            rstds, nmrs = emit_ln()
            xnT = emit_xnT(rstds, nmrs, guard=True)
            wq = load_w("wqs", "wq")
            wk = load_w("wks", "wk")
            wv = load_w("wvs", "wv")
            wo = load_w("wos", "wo")

            qT = bpool.tile([P, H, NTOK], bf16, tag="big", name="qT")
            kT = bpool.tile([P, H, NTOK + 2 * GK], bf16, tag="big", name="kT")
            for h in range(H):
                nc.gpsimd.memset(kT[:, h, 0:GK], 0.0)
                nc.gpsimd.memset(kT[:, h, GK + NTOK:], 0.0)
            for h in range(H):
                for t4 in range(NT4):
                    tok0 = t4 * T4
                    psq = ppool.tile([P, T4], f32, tag="pbig", name="psq")
                    for dcc in range(DC):
                        nc.tensor.matmul(
                            psq[:], wq[:, dcc, h * DH:(h + 1) * DH],
                            xnT[:, dcc, GK + tok0: GK + tok0 + T4],
                            start=(dcc == 0), stop=(dcc == DC - 1))
                    nc.vector.tensor_scalar(
                        out=qT[:, h, tok0:tok0 + T4], in0=psq[:],
                        scalar1=cs["bqc"][:, h:h + 1], scalar2=None,
                        op0=ALU.add)
                    psk = ppool.tile([P, T4], f32, tag="pbig", name="psk")
                    for dcc in range(DC):
                        nc.tensor.matmul(
                            psk[:], wk[:, dcc, h * DH:(h + 1) * DH],
                            xnT[:, dcc, GK + tok0: GK + tok0 + T4],
                            start=(dcc == 0), stop=(dcc == DC - 1))
                    nc.vector.tensor_scalar(
                        out=kT[:, h, GK + tok0: GK + tok0 + T4], in0=psk[:],
                        scalar1=cs["bkc"][:, h:h + 1], scalar2=None,
                        op0=ALU.add)

            # shifted v tiles: tile st covers tokens [st*128-6, st*128+122)
            v_sb = bpool.tile([P, NT, H * DH], bf16, tag="big", name="v_sb")
            v17 = spool.tile([P, H * DH], bf16, tag="v17", bufs=1, name="v17")
            for st in range(NT + 1):
                m = P if st < NT else GK
                ps = ppool.tile([P, H * DH], f32, tag="pbig", name="psv")
                c0 = GK + st * P - WF
                for dcc in range(DC):
                    nc.tensor.matmul(
                        ps[:m, :], xnT[:, dcc, c0:c0 + m], wv[:, dcc, :],
                        start=(dcc == 0),
                        stop=(zero_bias and dcc == DC - 1))
                if not zero_bias:
                    nc.tensor.matmul(ps[:m, :], cs["onesr"][:, 0:m],
                                     cs["bvr"][:], start=False, stop=True)
                dst = v_sb[:, st, :] if st < NT else v17[0:GK, :]
                nc.vector.tensor_copy(dst, ps[:m, :])

            for st in range(NT):
                i0 = st * P
                tin = st % TPS
                mi = 0 if tin == 0 else (2 if tin == TPS - 1 else 1)
                oT_t = spool.tile([P, H, P], bf16, tag="oTt", bufs=2,
                                  name="oT_t")
                for hp in range(H // 2):
                    sps = pspool.tile([P, 2, WIN], f32, tag="psc", bufs=1,
                                      name="sps")
                    for hh in range(2):
                        h = hp * 2 + hh
                        nc.tensor.matmul(
                            sps[:, hh, :], qT[:, h, i0:i0 + P],
                            kT[:, h, GK + i0 - WF: GK + i0 - WF + WIN],
                            start=True, stop=True)
                    sm = spool.tile([P, 2, WIN], f32, tag="sm", name="sm")
                    for hh in range(2):
                        nc.vector.tensor_tensor(
                            out=sm[:, hh, :], in0=sps[:, hh, :],
                            in1=cs["masks"][:, mi, :], op=ALU.add)
                    for hh in range(2):
                        h = hp * 2 + hh
                        ex = spool.tile([P, WIN], bf16, tag="ex", name="ex")
                        den = spool.tile([P, 1], f32, tag="den", name="den")
                        nc.scalar.activation(
                            out=ex[:], in_=sm[:, hh, :], func=AF.Exp,
                            bias=0.0, accum_out=den[:])
                        rden = spool.tile([P, 1], f32, tag="rden", name="rden")
                        nc.vector.reciprocal(out=rden[:], in_=den[:])
                        at = spool.tile([P, WIN], bf16, tag="at", name="at")
                        nc.vector.tensor_scalar_mul(at[:], ex[:], rden[:])
                        pt2 = pspool.tile([P, 2 * P], bf16, tag="pt2", bufs=2,
                                          name="pt2")
                        nc.tensor.transpose(pt2[:, 0:P], at[:, 0:P],
                                            cs["ident"][:])
                        nc.tensor.transpose(pt2[0:GK, P:P + P], at[:, P:WIN],
                                            cs["ident"][:])
                        aT = spool.tile([P, P], bf16, tag="aT", name="aT")
                        bT = spool.tile([GK, P], bf16, tag="bT", name="bT")
                        nc.scalar.activation(out=aT[:], in_=pt2[:, 0:P],
                                             func=AF.Copy)
                        nc.vector.tensor_copy(bT[:], pt2[0:GK, P:P + P])
                        po = pspool.tile([P, P], f32, tag="pav", bufs=1,
                                         name="po")
                        nc.tensor.matmul(po[:],
                                         v_sb[:, st, h * DH:(h + 1) * DH],
                                         aT[:], start=True, stop=False)
                        vn = (v_sb[0:GK, st + 1, h * DH:(h + 1) * DH]
                              if st + 1 < NT else v17[0:GK, :][:, h * DH:(h + 1) * DH])
                        nc.tensor.matmul(po[:, P - GK:P], vn,
                                         bT[:, P - GK:P],
                                         start=False, stop=True)
                        nc.scalar.activation(out=oT_t[:, h, :], in_=po[:],
                                             func=AF.Copy)
                # output projection + residual for this tile
                pso = ppool.tile([P, D], f32, tag="pbig", name="pso")
                for h in range(H):
                    nc.tensor.matmul(pso[:], oT_t[:, h, :], wo[:, h, :],
                                     start=(h == 0),
                                     stop=(zero_bias and h == H - 1))
                if not zero_bias:
                    nc.tensor.matmul(pso[:], cs["onesr"][:], cs["bor"][:],
                                     start=False, stop=True)
                nc.vector.tensor_tensor(out=X[:, st, :], in0=pso[:],
                                        in1=X[:, st, :], op=ALU.add)

        def conv_module():
            rstds, nmrs = emit_ln()
            xnT = emit_xnT(rstds, nmrs, dt=f8, tag="xnT8")
            pw1 = load_w("pw1s", "pw1")
            dgall = load_w("dgall", "dgall")
            gluT = bpool.tile([P, DC, 2, SL], f8, tag="big", name="gluT")
            for dcc in range(DC):
                for s in range(2):
                    nc.gpsimd.memset(gluT[:, dcc, s, 0:GC], 0.0)
            for fc in range(DC):
                for t4 in range(NT4):
                    tok0 = t4 * T4
                    s, hf = t4 // 2, t4 % 2
                    psa = ppool.tile([P, T4], f32, tag="pbig", name="psa")
                    psg = ppool.tile([P, T4], f32, tag="pbig", name="psg")
                    for c in range(DC // 2):
                        nc.tensor.matmul(
                            psa[:], pw1[:, 2 * c:2 * c + 2,
                                        fc * P:(fc + 1) * P],
                            xnT[:, 2 * c:2 * c + 2, GK + tok0: GK + tok0 + T4],
                            start=(c == 0), stop=(c == DC // 2 - 1),
                            perf_mode=DR)
                    for c in range(DC // 2):
                        nc.tensor.matmul(
                            psg[:], pw1[:, 2 * c:2 * c + 2,
                                        D + fc * P: D + (fc + 1) * P],
                            xnT[:, 2 * c:2 * c + 2, GK + tok0: GK + tok0 + T4],
                            start=(c == 0), stop=(c == DC // 2 - 1),
                            perf_mode=DR)
                    sg = spool.tile([P, T4], bf16, tag="sg", name="sg")
                    nc.scalar.activation(out=sg[:], in_=psg[:], func=AF.Sigmoid,
                                         bias=cs["pw1bc"][:, DC + fc:DC + fc + 1],
                                         scale=1.0 / WS)
                    # gluT = (psa + WS*b_a) * sg = WS * glu_true  (fp8)
                    nc.vector.scalar_tensor_tensor(
                        out=gluT[:, fc, s, GC + hf * T4: GC + (hf + 1) * T4],
                        in0=psa[:], scalar=cs["pw1bc"][:, fc:fc + 1], in1=sg[:],
                        op0=ALU.add, op1=ALU.mult)

            sep = load_w("seps", "sep")
            convT = bpool.tile([P, DC, 2, L], f8, tag="big", name="convT")
            # depthwise conv: 16 paired-tap fp8 DoubleRow diagonal matmuls
            # per (chunk, 512-token tile); rhs pairs taps (k, k+16) via a
            # stride-16 middle AP dim over the shifted glu slice.
            for dcc in range(DC):
                for t4 in range(NT4):
                    s, hf = t4 // 2, t4 % 2
                    psc_ = ppool.tile([P, T4], f32, tag="pbig", name="psc_")
                    for kp in range(KC // 2):
                        rhs = gluT[:, dcc, s,
                                   1 + hf * T4 + kp: 1 + hf * T4 + kp + T4]
                        rhs = rhs.unsqueeze(1)
                        rhs.ap[1] = (16, 2)
                        nc.tensor.matmul(
                            psc_[:], dgall[:, dcc, kp, :, :], rhs,
                            start=(kp == 0), stop=(kp == KC // 2 - 1),
                            perf_mode=DR)
                    # convT = SCV * conv_true (fp8)
                    nc.vector.tensor_scalar(
                        out=convT[:, dcc, s, hf * T4:(hf + 1) * T4],
                        in0=psc_[:], scalar1=CS, scalar2=None, op0=ALU.mult)

            pw2 = load_w("pw2s", "pw2")
            for t4 in range(NT4):
                tok0 = t4 * T4
                s, hf = t4 // 2, t4 % 2
                silT = bpool.tile([P, 2 * DC, T4], f8, tag="big",
                                  name="silT")
                for fc in range(2 * DC):
                    ps = ppool.tile([P, T4], f32, tag="pbig", name="pss1")
                    for c in range(DC // 2):
                        nc.tensor.matmul(
                            ps[:], sep[:, 2 * c:2 * c + 2,
                                       fc * P:(fc + 1) * P],
                            convT[:, 2 * c:2 * c + 2, s,
                                  hf * T4:(hf + 1) * T4],
                            start=(c == 0), stop=(c == DC // 2 - 1),
                            perf_mode=DR)
                    emit_silu(silT[:, fc, :], ps[:], cs["sepbc"][:, fc:fc + 1],
                              scale=1.0 / (SCV * WS))
                for j in range(T4 // P):
                    t = (tok0 // P) + j
                    ps2 = ppool.tile([P, D], f32, tag="pbig", name="pss2")
                    for c in range(DC):
                        nc.tensor.matmul(
                            ps2[:], silT[:, 2 * c:2 * c + 2,
                                         j * P:(j + 1) * P],
                            pw2[:, 2 * c:2 * c + 2, :], start=(c == 0),
                            stop=(zero_bias and c == DC - 1),
                            perf_mode=DR)
                    if not zero_bias:
                        # pw2br host-scaled by WS
                        nc.tensor.matmul(ps2[:], cs["onesr"][:],
                                         cs["pw2br"][:],
                                         start=False, stop=True)
                    nc.vector.scalar_tensor_tensor(
                        out=X[:, t, :], in0=ps2[:], scalar=1.0 / WS,
                        in1=X[:, t, :], op0=ALU.mult, op1=ALU.add)

        def dump_dbg(i):
            if dbg:
                for t in range(NT):
                    Xc = spool.tile([P, D], f32, tag="dbgc", bufs=2,
                                    name="Xc")
                    nc.vector.tensor_copy(Xc[:], X[:, t, :])
                    nc.scalar.dma_start(
                        out=dbg[i].rearrange("(t p) d -> p t d", p=P)[:, t, :],
                        in_=Xc[:])

        # ---- pipeline -----------------------------------------------------
        if stages & 1:
            w1a = load_w("w1a", "w1", engs=(nc.gpsimd, nc.tensor))
            w2a = load_w("w2a", "w2", engs=(nc.gpsimd, nc.tensor))
            ff_module(w1a, cs["b1ca"], w2a, cs["b2ra"])
            dump_dbg(0)
        if stages & 2:
            mhsa_module()
            dump_dbg(1)
        if stages & 4:
            conv_module()
            dump_dbg(2)
        if stages & 8:
            w1b = load_w("w1b", "w1")
            w2b = load_w("w2b", "w2")
            ff_module(w1b, cs["b1cb"], w2b, cs["b2rb"])
            dump_dbg(3)

        # ---- final LN + store --------------------------------------------
        rstds, nmrs = emit_ln()
        for t in range(NT):
            xo = spool.tile([P, D], f32, tag="xo", name="xo")
            nc.vector.tensor_scalar(
                out=xo[:], in0=X[:, t, :],
                scalar1=rstds[t // 4][:, t % 4:t % 4 + 1],
                scalar2=nmrs[t // 4][:, t % 4:t % 4 + 1],
                op0=ALU.mult, op1=ALU.add)
            if not trivial_final_gb:
                nc.vector.tensor_tensor(out=xo[:], in0=xo[:],
                                        in1=cs["lngr"][:], op=ALU.mult)
                nc.vector.tensor_tensor(out=xo[:], in0=xo[:],
                                        in1=cs["lnbr"][:], op=ALU.add)
            nc.scalar.dma_start(
                out=y_d.rearrange("(t p) d -> p t d", p=P)[:, t, :], in_=xo[:])


# ---------------------------------------------------------------------------
_cache = {}


def get_nc(debug_stages=False, trivial_final_gb=True, sim_safe=False,
           stages=15, zero_bias=False):
    key = ("nc", debug_stages, trivial_final_gb, sim_safe, stages, zero_bias)
    if key not in _cache:
        _install_hook()
        _cache[key] = build_nc(debug_stages, trivial_final_gb, sim_safe,
                               stages, zero_bias)
    return _cache[key]


def make_in_maps(inputs, debug_stages=False):
    prep = host_prep(inputs)
    x = np.asarray(inputs["inputs"], np.float32)
    trivial = (np.all(np.asarray(inputs["ln_g"]) == 1.0)
               and np.all(np.asarray(inputs["ln_b"]) == 0.0))
    zero_bias = all(
        not np.any(prep[k]) for k in ("b2ra", "b2rb", "bvr", "bor", "pw2br"))
    if not trivial:
        pass
    in_maps = []
    for c in range(NCORES):
        m = dict(prep)
        m["x"] = np.ascontiguousarray(x[c * BL:(c + 1) * BL].reshape(NTOK, D))
        in_maps.append(m)
    return in_maps, trivial, zero_bias


def kernel(**inputs):
    _install_hook()
    in_maps, trivial, zero_bias = make_in_maps(inputs)
    nc = get_nc(trivial_final_gb=trivial, zero_bias=zero_bias)
    res = run_bass_kernel_spmd(nc, in_maps, list(range(NCORES)))
    outs = [res.results[c]["y"].reshape(BL, L, D) for c in range(NCORES)]
    return np.concatenate(outs, axis=0)



# revision 32
# speedup vs baseline: 1.3594x; 1.0013x over previous
"""ChunkConformerBlock Trainium2 kernel.

Full inputs -> full output. Data-parallel over batch: B=16 sequences split
2-per-core across 8 NeuronCores; all parameters replicated. Each core runs
the whole conformer block (ff1 -> banded MHSA -> conv -> ff2 -> final LN)
on its 2048 tokens with activations kept in SBUF.

Layout strategy per core:
  - residual stream X: token-major fp32 tiles [128 tok, 512 d] (16 tiles)
  - LayerNorm: bn_stats/bn_aggr on DVE (token-major), fused scale+shift apply
  - matmul internals: feature-major bf16 (xn^T via PE transposes); weights
    pre-transposed host-side into [128, kchunks, N] bf16 SBUF layouts with
    LN gammas/betas, BN affine and 1/sqrt(dh) folded in
  - banded attention (band [i-6, i+2]): 136-wide key windows from guarded
    feature-major k^T; softmax token-major; attn transposed on PE for the
    attn@V matmul; shifted token-major v tiles so all matmul operands start
    at partition 0
  - depthwise causal conv K=32: 32 shifted fused multiply-adds
    (scalar_tensor_tensor) on DVE/GPSIMD over feature-major bf16 with
    zero guard columns per sequence
"""

import numpy as np
import ml_dtypes

import concourse.bass as bass
import concourse.tile as tile
from concourse import mybir
from concourse.bass_utils import run_bass_kernel_spmd

import json as _json

# ---------------------------------------------------------------------------
# walrus in this container supports at most ONE sync-wait command per
# instruction; Tile can emit more. Split extras onto standalone
# EventSemaphore instructions at the BIR-JSON level.
_wsplit_ctr = [0]


def _split_waits(bir_json: bytes, cap: int = 1) -> bytes:
    j = _json.loads(bir_json)
    changed = False
    for f in j.get("functions", []):
        for b in f.get("blocks", []):
            new_list = []
            for ins in b.get("instructions", []):
                si = ins.get("sync_info") or {}
                waits = si.get("on_wait") or []
                if len(waits) > cap:
                    extra, keep = waits[:-cap], waits[-cap:]
                    si["on_wait"] = keep
                    ins["sync_info"] = si
                    for w in extra:
                        _wsplit_ctr[0] += 1
                        new_list.append({
                            "engine": ins.get("engine"),
                            "ins": [], "outs": [],
                            "name": f"I-wsplit-{_wsplit_ctr[0]}",
                            "opcode": "EventSemaphore",
                            "sync_info": {"on_update": [], "on_wait": [w]},
                        })
                    changed = True
                new_list.append(ins)
            b["instructions"] = new_list
    return _json.dumps(j).encode() if changed else bir_json


_hook_installed = [False]


def _install_hook():
    if _hook_installed[0]:
        return
    from concourse import bass_utils as _bu, bass2jax as _b2j
    orig = _bu.compile_bir_kernel

    def patched(bir_json, tmpdir, neff_name="file.neff"):
        return orig(_split_waits(bir_json), tmpdir, neff_name)

    _bu.compile_bir_kernel = patched
    _b2j.compile_bir_kernel = patched
    _hook_installed[0] = True


# ---------------------------------------------------------------------------
f32 = mybir.dt.float32
bf16 = mybir.dt.bfloat16
f8 = mybir.dt.float8e4
AF = mybir.ActivationFunctionType
ALU = mybir.AluOpType
AX = mybir.AxisListType
DR = mybir.MatmulPerfMode.DoubleRow
bfnp = ml_dtypes.bfloat16
f8np = mybir.dt.np(mybir.dt.float8e4)
WS = 128.0                    # fp8 weight pre-scale (undone on psum read)
SCV = 64.0                    # dwconv output stored-scale (convT = 64*conv)
CS = SCV / (WS * WS)          # conv psum -> convT multiplier

NCORES = 8
B, L, D = 16, 1024, 512
BL = B // NCORES              # sequences per core
NTOK = BL * L                 # tokens per core
P = 128
NT = NTOK // P                # 16 token tiles
DC = D // P                   # 4 d-chunks
F = 4 * D                     # 2048 ff hidden
FCH = F // P                  # 16 f-chunks
H, DH = 4, 128
KC = 32                       # conv taps
WF, WB = 6, 2
WIN = 136                     # key window = 128 + WF + WB
FC = 0.5
EPS = 1e-3
GK = 8                        # kT/xnT guard columns each side
GC = 32                       # gluT guard columns (per sequence, front)
T4 = 512                      # wide token tile for N=512 matmuls
NT4 = NTOK // T4              # 4
SL = GC + L                   # per-seq glu row length
TPS = L // P                  # tiles per sequence (8)
DVE_TAPS = 32                 # dwconv taps on DVE (gpsimd lacks stt)


def _chunk_k(w, n_chunk):
    """[K, N] host weight -> [128, n_chunk, N] SBUF layout (K on partitions)."""
    K, N = w.shape
    assert K == n_chunk * P
    return np.ascontiguousarray(w.reshape(n_chunk, P, N).transpose(1, 0, 2))


def _bias_cols(b, n_chunk):
    """[N] bias -> [128, n_chunk] per-partition bias columns."""
    assert b.shape[0] == n_chunk * P
    return np.ascontiguousarray(b.reshape(n_chunk, P).T)


def _chunk_mask_np(n):
    i = np.arange(n)[:, None]
    j = np.arange(n)[None, :]
    low = np.maximum(i - WF, 0)
    high = np.clip(i + WB, 0, n)
    low = low - np.maximum(low - n + WB, 0)
    high = np.maximum(high, WB)
    return (j >= low) & (j <= high)


def host_prep(inputs):
    """Fold LN gammas/betas, BN affine, 1/sqrt(dh); build SBUF-layout arrays."""
    g = {k: np.asarray(v, np.float32) for k, v in inputs.items()}
    out = {}

    for pfx, tag in (("ff1", "a"), ("ff2", "b")):
        w1 = g[pfx + "_g"][:, None] * g[pfx + "_w1"]
        b1 = g[pfx + "_b"] @ g[pfx + "_w1"] + g[pfx + "_b1"]
        out["w1" + tag] = _chunk_k(w1 * WS, DC).astype(f8np)
        out["b1c" + tag] = _bias_cols(b1, FCH)
        out["w2" + tag] = _chunk_k(g[pfx + "_w2"] * WS, FCH).astype(f8np)
        out["b2r" + tag] = (g[pfx + "_b2"] * WS)[None, :].astype(bfnp)

    sc = 1.0 / np.sqrt(DH)
    for nm, scale in (("wq", sc), ("wk", 1.0), ("wv", 1.0)):
        wf = g[nm].reshape(D, H * DH)
        bf_ = g["b" + nm[1]].reshape(H * DH)
        wp = (g["mh_g"][:, None] * wf) * scale
        bp = (g["mh_b"] @ wf + bf_) * scale
        out[nm + "s"] = _chunk_k(wp * WS, DC).astype(f8np)
        if nm != "wv":
            out["b" + nm[1] + "c"] = _bias_cols(bp, H)
        else:
            out["bvr"] = (bp * WS)[None, :].astype(bfnp)
    out["wos"] = _chunk_k(g["wo"].reshape(H * DH, D) * WS, H).astype(f8np)
    out["bor"] = (g["bo"] * WS)[None, :].astype(bfnp)

    pw1 = g["cv_g"][:, None] * g["pw1_w"]
    pw1b = g["cv_b"] @ g["pw1_w"] + g["pw1_b"]
    out["pw1s"] = _chunk_k(pw1 * WS, DC).astype(f8np)
    # a-half bias pre-scaled by WS so gluT = WS * glu_true (one stt keeps
    # (psa + WS*b_a) * sigmoid); gate-half bias unscaled (applied after
    # scale=1/WS inside the Sigmoid activation).
    pw1b_sc = pw1b.copy()
    pw1b_sc[:D] *= WS
    out["pw1bc"] = _bias_cols(pw1b_sc, 2 * DC)
    s = g["bn_g"] / np.sqrt(g["bn_v"] + EPS)
    t = g["bn_b"] - g["bn_m"] * s
    sepw = g["sep_w"] * s[None, :]
    sepb = g["sep_b"] * s + t
    out["seps"] = _chunk_k(sepw * WS, DC).astype(f8np)
    out["sepbc"] = _bias_cols(sepb, 2 * DC)
    out["pw2s"] = _chunk_k(g["pw2_w"] * WS, 2 * DC).astype(f8np)
    out["pw2br"] = (g["pw2_b"] * WS)[None, :].astype(bfnp)

    out["dww"] = np.ascontiguousarray(
        g["dw_w"].T.reshape(DC, P, KC).transpose(1, 0, 2)).astype(np.float32)
    # paired diag taps for fp8 DoubleRow dwconv:
    # dgall[p, dcc, kp, i, m] = WS * dw_w[kp + 16*i, dcc*128+p] iff m == p
    dg = np.zeros((P, DC, KC // 2, 2, P), np.float32)
    for dcc in range(DC):
        for kp in range(KC // 2):
            for i in range(2):
                w = g["dw_w"][kp + 16 * i, dcc * P:(dcc + 1) * P] * WS
                dg[np.arange(P), dcc, kp, i, np.arange(P)] = w
    out["dgall"] = dg.astype(f8np)

    out["lngr"] = np.broadcast_to(g["ln_g"][None, :], (P, D)).astype(np.float32).copy()
    out["lnbr"] = np.broadcast_to(g["ln_b"][None, :], (P, D)).astype(np.float32).copy()

    m_full = _chunk_mask_np(L)
    masks = np.full((P, 3, WIN), -1e9, np.float32)
    for mi, t in ((0, 0), (1, 3), (2, TPS - 1)):
        i0 = t * P
        for r in range(P):
            for c in range(WIN):
                jj = i0 - WF + c
                if 0 <= jj < L and m_full[i0 + r, jj]:
                    masks[r, mi, c] = 0.0
    out["masks"] = masks
    out["masks2"] = np.repeat(masks[:, :, None, :], 2, axis=2).astype(bfnp)

    out["ident"] = np.eye(P, dtype=np.float32).astype(bfnp)
    out["onesr"] = np.ones((1, P), np.float32).astype(bfnp)
    out["b2ra32"] = out["b2ra"].astype(np.float32)
    out["b2rb32"] = out["b2rb"].astype(np.float32)
    out["bor32"] = out["bor"].astype(np.float32)
    out["pw2br32"] = out["pw2br"].astype(np.float32)
    out["bvr32"] = out["bvr"].astype(np.float32)
    return out


SPECS = {
    "w1a": ([P, DC, F], f8), "b1ca": ([P, FCH], f32),
    "w2a": ([P, FCH, D], f8), "b2ra": ([1, D], bf16),
    "w1b": ([P, DC, F], f8), "b1cb": ([P, FCH], f32),
    "w2b": ([P, FCH, D], f8), "b2rb": ([1, D], bf16),
    "wqs": ([P, DC, H * DH], f8), "bqc": ([P, H], f32),
    "wks": ([P, DC, H * DH], f8), "bkc": ([P, H], f32),
    "wvs": ([P, DC, H * DH], f8), "bvr": ([1, H * DH], bf16),
    "wos": ([P, H, D], f8), "bor": ([1, D], bf16),
    "pw1s": ([P, DC, 2 * D], f8), "pw1bc": ([P, 2 * DC], f32),
    "seps": ([P, DC, 2 * D], f8), "sepbc": ([P, 2 * DC], f32),
    "pw2s": ([P, 2 * DC, D], f8), "pw2br": ([1, D], bf16),
    "dww": ([P, DC, KC], f32),
    "dgall": ([P, DC, KC // 2, 2, P], f8),
    "lngr": ([P, D], f32), "lnbr": ([P, D], f32),
    "masks": ([P, 3, WIN], f32),
    "masks2": ([P, 3, 2, WIN], bf16),
    "ident": ([P, P], bf16), "onesr": ([1, P], bf16),
    "b2ra32": ([1, D], f32), "b2rb32": ([1, D], f32),
    "bor32": ([1, D], f32), "pw2br32": ([1, D], f32),
    "bvr32": ([1, H * DH], f32),
}


# ---------------------------------------------------------------------------
def build_nc(debug_stages=False, trivial_final_gb=True, sim_safe=False, stages=15, zero_bias=False):
    nc = bass.Bass()

    x_d = nc.dram_tensor("x", [NTOK, D], f32, kind="ExternalInput")
    y_d = nc.dram_tensor("y", [NTOK, D], f32, kind="ExternalOutput")
    dbg = []
    if debug_stages:
        for i in range(4):
            dbg.append(nc.dram_tensor(f"dbg{i}", [NTOK, D], f32,
                                      kind="ExternalOutput"))

    wd = {}
    for nm, (shp, dt) in SPECS.items():
        wd[nm] = nc.dram_tensor(nm, shp, dt, kind="ExternalInput")

    with tile.TileContext(nc) as tc:
        _emit(nc, tc, x_d, y_d, wd, dbg, trivial_final_gb, sim_safe, stages, zero_bias)
    return nc


def _emit(nc, tc, x_d, y_d, wd, dbg, trivial_final_gb, sim_safe, stages=15, zero_bias=False):
    from contextlib import ExitStack
    ctx = ExitStack()
    with ctx:
        cpool = ctx.enter_context(tc.tile_pool(name="const", bufs=1))
        wpool = ctx.enter_context(tc.tile_pool(name="wts", bufs=1))
        xpool = ctx.enter_context(tc.tile_pool(name="xres", bufs=1))
        apool = ctx.enter_context(tc.tile_pool(name="acts", bufs=1))
        bpool = ctx.enter_context(tc.tile_pool(name="big", bufs=3))
        spool = ctx.enter_context(tc.tile_pool(name="small", bufs=2))
        stpool = ctx.enter_context(tc.tile_pool(name="stats", bufs=2))
        ppool = ctx.enter_context(tc.tile_pool(name="ps", bufs=3, space="PSUM"))
        pspool = ctx.enter_context(tc.tile_pool(name="pss", bufs=1, space="PSUM"))

        # ---- X residual stream (split across queues; tiles 0-3 first) ----
        dmaengs = [nc.sync, nc.scalar, nc.gpsimd]
        X = xpool.tile([P, NT, D], f32, tag="X", name="X")
        xr = x_d.rearrange("(t p) d -> p t d", p=P)
        for t in range(NT):
            dmaengs[t % 3].dma_start(out=X[:, t, :], in_=xr[:, t, :])

        def load_w(nm, tag, engs=(nc.gpsimd,)):
            shp, dt = SPECS[nm]
            t = wpool.tile(shp, dt, tag=tag, name=nm + "_sb")
            if len(shp) == 3 and shp[1] > 1:
                for c in range(shp[1]):
                    engs[c % len(engs)].dma_start(
                        out=t[:, c, :], in_=wd[nm][:, c, :])
            else:
                engs[0].dma_start(out=t[:], in_=wd[nm][:])
            return t

        # ---- persistent consts -------------------------------------------
        cs = {}
        cnames = ["b1ca", "b2ra", "b1cb", "b2rb", "bqc", "bkc", "bvr",
                  "bor", "pw1bc", "sepbc", "pw2br",
                  "masks", "ident", "onesr"]
        if not trivial_final_gb:
            cnames += ["lngr", "lnbr"]
        for nm in cnames:
            shp, dt = SPECS[nm]
            t = cpool.tile(shp, dt, tag=nm, name=nm)
            eng = nc.sync if nm in ("ident",) else nc.scalar
            eng.dma_start(out=t[:], in_=wd[nm][:])
            cs[nm] = t
        eps_t = cpool.tile([P, 1], f32, tag="eps", name="eps_t")
        nc.vector.memset(eps_t[:], EPS)

        # ---- helpers ------------------------------------------------------
        def bias_accum(ps_ap, nm, width, rows=P):
            nc.gpsimd.dma_start(
                out=ps_ap, in_=wd[nm][:].to_broadcast([rows, width]),
                accum_op=ALU.add)

        def emit_silu(out_ap, psum_ap, bias_ap, scale=1.0):
            nc.scalar.activation(out=out_ap, in_=psum_ap, func=AF.Silu,
                                 bias=bias_ap, scale=scale)

        def emit_ln():
            rstds, nmrs = [], []
            for g in range(NT // 4):
                mv = stpool.tile([P, 4, 2], f32, tag="mv", bufs=3, name="mv")
                for j in range(4):
                    st6 = stpool.tile([P, 6], f32, tag="st6", bufs=3,
                                      name="st6")
                    nc.vector.bn_stats(out=st6[:], in_=X[:, g * 4 + j, :])
                    nc.vector.bn_aggr(out=mv[:, j, :], in_=st6[:])
                rstd = stpool.tile([P, 4], f32, tag="rstd", bufs=3,
                                   name="rstd")
                nmr = stpool.tile([P, 4], f32, tag="nmr", bufs=3, name="nmr")
                nc.scalar.activation(out=rstd[:], in_=mv[:, :, 1],
                                     func=AF.Sqrt, bias=eps_t[:])
                nc.vector.reciprocal(out=rstd[:], in_=rstd[:])
                nc.vector.scalar_tensor_tensor(
                    out=nmr[:], in0=mv[:, :, 0], scalar=-1.0, in1=rstd[:],
                    op0=ALU.mult, op1=ALU.mult)
                rstds.append(rstd)
                nmrs.append(nmr)
            return rstds, nmrs

        def emit_xnT(rstds, nmrs, guard=False, dt=bf16, tag="xnT"):
            xnT = apool.tile([P, DC, NTOK + 2 * GK], dt, tag=tag,
                             name=tag)
            if guard:
                for dcc in range(DC):
                    nc.gpsimd.memset(xnT[:, dcc, 0:GK], 0.0)
                    nc.gpsimd.memset(xnT[:, dcc, GK + NTOK:], 0.0)
            for t in range(NT):
                xn = spool.tile([P, D], bf16, tag="xn", bufs=3, name="xn")
                nc.vector.tensor_scalar(
                    out=xn[:], in0=X[:, t, :],
                    scalar1=rstds[t // 4][:, t % 4:t % 4 + 1],
                    scalar2=nmrs[t // 4][:, t % 4:t % 4 + 1],
                    op0=ALU.mult, op1=ALU.add)
                pt = pspool.tile([P, D], bf16, tag="ptr", bufs=1, name="pt")
                for dcc in range(DC):
                    nc.tensor.transpose(pt[:, dcc * P:(dcc + 1) * P],
                                        xn[:, dcc * P:(dcc + 1) * P],
                                        cs["ident"][:])
                nc.scalar.activation(
                    out=xnT[:, :, GK + t * P: GK + (t + 1) * P],
                    in_=pt[:].rearrange("p (c t) -> p c t", c=DC),
                    func=AF.Copy)
            return xnT

        def ff_module(w1, b1c, w2, b2r):
            rstds, nmrs = emit_ln()
            xnT = emit_xnT(rstds, nmrs, dt=f8, tag="xnT8")
            for t4 in range(NT4):
                tok0 = t4 * T4
                h1T = bpool.tile([P, FCH, T4], f8, tag="big", name="h1T")
                for fc in range(FCH):
                    ps = ppool.tile([P, T4], f32, tag="pbig", name="psf")
                    for c in range(DC // 2):
                        nc.tensor.matmul(
                            ps[:], w1[:, 2 * c:2 * c + 2, fc * P:(fc + 1) * P],
                            xnT[:, 2 * c:2 * c + 2, GK + tok0: GK + tok0 + T4],
                            start=(c == 0), stop=(c == DC // 2 - 1),
                            perf_mode=DR)
                    emit_silu(h1T[:, fc, :], ps[:], b1c[:, fc:fc + 1],
                              scale=1.0 / WS)
                for j in range(T4 // P):
                    t = (tok0 // P) + j
                    ps2 = ppool.tile([P, D], f32, tag="pbig", name="psb")
                    for c in range(FCH // 2):
                        nc.tensor.matmul(
                            ps2[:], h1T[:, 2 * c:2 * c + 2, j * P:(j + 1) * P],
                            w2[:, 2 * c:2 * c + 2, :],
                            start=(c == 0),
                            stop=(zero_bias and c == FCH // 2 - 1),
                            perf_mode=DR)
                    if not zero_bias:
                        # b2r host-scaled by WS so the FC/WS unscale is right
                        nc.tensor.matmul(ps2[:], cs["onesr"][:], b2r[:],
                                         start=False, stop=True)
                    nc.vector.scalar_tensor_tensor(
                        out=X[:, t, :], in0=ps2[:], scalar=FC / WS,
                        in1=X[:, t, :], op0=ALU.mult, op1=ALU.add)

        def mhsa_module():
            rstds, nmrs = emit_ln()
            xnT = emit_xnT(rstds, nmrs, guard=True, dt=f8, tag="xnT8")
            wq = load_w("wqs", "wq")
            wk = load_w("wks", "wk")
            wv = load_w("wvs", "wv")
            wo = load_w("wos", "wo")

            qT = bpool.tile([P, H, NTOK], bf16, tag="big", name="qT")
            kT = bpool.tile([P, H, NTOK + 2 * GK], bf16, tag="big", name="kT")
            for h in range(H):
                nc.gpsimd.memset(kT[:, h, 0:GK], 0.0)
                nc.gpsimd.memset(kT[:, h, GK + NTOK:], 0.0)
            for h in range(H):
                for t4 in range(NT4):
                    tok0 = t4 * T4
                    psq = ppool.tile([P, T4], f32, tag="pbig", name="psq")
                    for c in range(DC // 2):
                        nc.tensor.matmul(
                            psq[:], wq[:, 2 * c:2 * c + 2,
                                       h * DH:(h + 1) * DH],
                            xnT[:, 2 * c:2 * c + 2, GK + tok0: GK + tok0 + T4],
                            start=(c == 0), stop=(c == DC // 2 - 1),
                            perf_mode=DR)
                    nc.vector.tensor_scalar(
                        out=qT[:, h, tok0:tok0 + T4], in0=psq[:],
                        scalar1=1.0 / WS, scalar2=cs["bqc"][:, h:h + 1],
                        op0=ALU.mult, op1=ALU.add)
                    psk = ppool.tile([P, T4], f32, tag="pbig", name="psk")
                    for c in range(DC // 2):
                        nc.tensor.matmul(
                            psk[:], wk[:, 2 * c:2 * c + 2,
                                       h * DH:(h + 1) * DH],
                            xnT[:, 2 * c:2 * c + 2, GK + tok0: GK + tok0 + T4],
                            start=(c == 0), stop=(c == DC // 2 - 1),
                            perf_mode=DR)
                    nc.vector.tensor_scalar(
                        out=kT[:, h, GK + tok0: GK + tok0 + T4], in0=psk[:],
                        scalar1=1.0 / WS, scalar2=cs["bkc"][:, h:h + 1],
                        op0=ALU.mult, op1=ALU.add)

            # shifted v tiles: tile st covers tokens [st*128-6, st*128+122)
            v_sb = bpool.tile([P, NT, H * DH], bf16, tag="big", name="v_sb")
            v17 = spool.tile([P, H * DH], bf16, tag="v17", bufs=1, name="v17")
            for st in range(NT + 1):
                m = P if st < NT else GK
                ps = ppool.tile([P, H * DH], f32, tag="pbig", name="psv")
                c0 = GK + st * P - WF
                for c in range(DC // 2):
                    lhsT = xnT[:, 2 * c:2 * c + 2, c0:c0 + m]
                    nc.tensor.matmul(
                        ps[:m, :], lhsT, wv[:, 2 * c:2 * c + 2, :],
                        start=(c == 0),
                        stop=(zero_bias and c == DC // 2 - 1),
                        perf_mode=DR)
                if not zero_bias:
                    nc.tensor.matmul(ps[:m, :], cs["onesr"][:, 0:m],
                                     cs["bvr"][:], start=False, stop=True)
                dst = v_sb[:, st, :] if st < NT else v17[0:GK, :]
                nc.vector.tensor_scalar(dst, in0=ps[:m, :],
                                        scalar1=1.0 / WS, scalar2=None,
                                        op0=ALU.mult)

            for st in range(NT):
                i0 = st * P
                tin = st % TPS
                mi = 0 if tin == 0 else (2 if tin == TPS - 1 else 1)
                oT_t = spool.tile([P, H, P], f8, tag="oTt", bufs=2,
                                  name="oT_t")
                for hp in range(H // 2):
                    sps = pspool.tile([P, 2, WIN], f32, tag="psc", bufs=2,
                                      name="sps")
                    for hh in range(2):
                        h = hp * 2 + hh
                        nc.tensor.matmul(
                            sps[:, hh, :], qT[:, h, i0:i0 + P],
                            kT[:, h, GK + i0 - WF: GK + i0 - WF + WIN],
                            start=True, stop=True)
                    sm = spool.tile([P, 2, WIN], f32, tag="sm", name="sm")
                    for hh in range(2):
                        nc.vector.tensor_tensor(
                            out=sm[:, hh, :], in0=sps[:, hh, :],
                            in1=cs["masks"][:, mi, :], op=ALU.add)
                    for hh in range(2):
                        h = hp * 2 + hh
                        ex = spool.tile([P, WIN], bf16, tag="ex", name="ex")
                        den = spool.tile([P, 1], f32, tag="den", name="den")
                        nc.scalar.activation(
                            out=ex[:], in_=sm[:, hh, :], func=AF.Exp,
                            bias=0.0, accum_out=den[:])
                        rden = spool.tile([P, 1], f32, tag="rden", name="rden")
                        nc.vector.reciprocal(out=rden[:], in_=den[:])
                        at = spool.tile([P, WIN], bf16, tag="at", name="at")
                        nc.vector.tensor_scalar_mul(at[:], ex[:], rden[:])
                        pt2 = pspool.tile([P, 2 * P], bf16, tag="pt2", bufs=1,
                                          name="pt2")
                        nc.tensor.transpose(pt2[:, 0:P], at[:, 0:P],
                                            cs["ident"][:])
                        nc.tensor.transpose(pt2[0:GK, P:P + P], at[:, P:WIN],
                                            cs["ident"][:])
                        aT = spool.tile([P, P], bf16, tag="aT", name="aT")
                        bT = spool.tile([GK, P], bf16, tag="bT", name="bT")
                        nc.scalar.activation(out=aT[:], in_=pt2[:, 0:P],
                                             func=AF.Copy)
                        nc.vector.tensor_copy(bT[:], pt2[0:GK, P:P + P])
                        po = pspool.tile([P, P], f32, tag="pav", bufs=1,
                                         name="po")
                        nc.tensor.matmul(po[:],
                                         v_sb[:, st, h * DH:(h + 1) * DH],
                                         aT[:], start=True, stop=False)
                        vn = (v_sb[0:GK, st + 1, h * DH:(h + 1) * DH]
                              if st + 1 < NT else v17[0:GK, :][:, h * DH:(h + 1) * DH])
                        nc.tensor.matmul(po[:, P - GK:P], vn,
                                         bT[:, P - GK:P],
                                         start=False, stop=True)
                        nc.scalar.activation(out=oT_t[:, h, :], in_=po[:],
                                             func=AF.Copy)
                # output projection + residual for this tile
                pso = ppool.tile([P, D], f32, tag="pbig", name="pso")
                for i in range(H // 2):
                    nc.tensor.matmul(pso[:], oT_t[:, 2 * i:2 * i + 2, :],
                                     wo[:, 2 * i:2 * i + 2, :],
                                     start=(i == 0),
                                     stop=(zero_bias and i == H // 2 - 1),
                                     perf_mode=DR)
                if not zero_bias:
                    nc.tensor.matmul(pso[:], cs["onesr"][:], cs["bor"][:],
                                     start=False, stop=True)
                nc.vector.scalar_tensor_tensor(
                    out=X[:, st, :], in0=pso[:], scalar=1.0 / WS,
                    in1=X[:, st, :], op0=ALU.mult, op1=ALU.add)

        def conv_module():
            rstds, nmrs = emit_ln()
            xnT = emit_xnT(rstds, nmrs, dt=f8, tag="xnT8")
            pw1 = load_w("pw1s", "pw1")
            dgall = load_w("dgall", "dgall")
            gluT = bpool.tile([P, DC, 2, SL], f8, tag="big", name="gluT")
            for dcc in range(DC):
                for s in range(2):
                    nc.gpsimd.memset(gluT[:, dcc, s, 0:GC], 0.0)
            for fc in range(DC):
                for t4 in range(NT4):
                    tok0 = t4 * T4
                    s, hf = t4 // 2, t4 % 2
                    psa = ppool.tile([P, T4], f32, tag="pbig", name="psa")
                    psg = ppool.tile([P, T4], f32, tag="pbig", name="psg")
                    for c in range(DC // 2):
                        nc.tensor.matmul(
                            psa[:], pw1[:, 2 * c:2 * c + 2,
                                        fc * P:(fc + 1) * P],
                            xnT[:, 2 * c:2 * c + 2, GK + tok0: GK + tok0 + T4],
                            start=(c == 0), stop=(c == DC // 2 - 1),
                            perf_mode=DR)
                    for c in range(DC // 2):
                        nc.tensor.matmul(
                            psg[:], pw1[:, 2 * c:2 * c + 2,
                                        D + fc * P: D + (fc + 1) * P],
                            xnT[:, 2 * c:2 * c + 2, GK + tok0: GK + tok0 + T4],
                            start=(c == 0), stop=(c == DC // 2 - 1),
                            perf_mode=DR)
                    sg = spool.tile([P, T4], bf16, tag="sg", name="sg")
                    nc.scalar.activation(out=sg[:], in_=psg[:], func=AF.Sigmoid,
                                         bias=cs["pw1bc"][:, DC + fc:DC + fc + 1],
                                         scale=1.0 / WS)
                    # gluT = (psa + WS*b_a) * sg = WS * glu_true  (fp8)
                    nc.vector.scalar_tensor_tensor(
                        out=gluT[:, fc, s, GC + hf * T4: GC + (hf + 1) * T4],
                        in0=psa[:], scalar=cs["pw1bc"][:, fc:fc + 1], in1=sg[:],
                        op0=ALU.add, op1=ALU.mult)

            sep = load_w("seps", "sep")
            convT = bpool.tile([P, DC, 2, L], f8, tag="big", name="convT")
            # depthwise conv: 16 paired-tap fp8 DoubleRow diagonal matmuls
            # per (chunk, 512-token tile); rhs pairs taps (k, k+16) via a
            # stride-16 middle AP dim over the shifted glu slice.
            for t4 in range(NT4):
                for dcc in range(DC):
                    s, hf = t4 // 2, t4 % 2
                    psc_ = ppool.tile([P, T4], f32, tag="pbig", name="psc_")
                    for kp in range(KC // 2):
                        rhs = gluT[:, dcc, s,
                                   1 + hf * T4 + kp: 1 + hf * T4 + kp + T4]
                        rhs = rhs.unsqueeze(1)
                        rhs.ap[1] = (16, 2)
                        nc.tensor.matmul(
                            psc_[:], dgall[:, dcc, kp, :, :], rhs,
                            start=(kp == 0), stop=(kp == KC // 2 - 1),
                            perf_mode=DR)
                    # convT = SCV * conv_true (fp8)
                    nc.vector.tensor_scalar(
                        out=convT[:, dcc, s, hf * T4:(hf + 1) * T4],
                        in0=psc_[:], scalar1=CS, scalar2=None, op0=ALU.mult)

            pw2 = load_w("pw2s", "pw2")
            for t4 in range(NT4):
                tok0 = t4 * T4
                s, hf = t4 // 2, t4 % 2
                silT = bpool.tile([P, 2 * DC, T4], f8, tag="big",
                                  name="silT")
                for fc in range(2 * DC):
                    ps = ppool.tile([P, T4], f32, tag="pbig", name="pss1")
                    for c in range(DC // 2):
                        nc.tensor.matmul(
                            ps[:], sep[:, 2 * c:2 * c + 2,
                                       fc * P:(fc + 1) * P],
                            convT[:, 2 * c:2 * c + 2, s,
                                  hf * T4:(hf + 1) * T4],
                            start=(c == 0), stop=(c == DC // 2 - 1),
                            perf_mode=DR)
                    emit_silu(silT[:, fc, :], ps[:], cs["sepbc"][:, fc:fc + 1],
                              scale=1.0 / (SCV * WS))
                for j in range(T4 // P):
                    t = (tok0 // P) + j
                    ps2 = ppool.tile([P, D], f32, tag="pbig", name="pss2")
                    for c in range(DC):
                        nc.tensor.matmul(
                            ps2[:], silT[:, 2 * c:2 * c + 2,
                                         j * P:(j + 1) * P],
                            pw2[:, 2 * c:2 * c + 2, :], start=(c == 0),
                            stop=(zero_bias and c == DC - 1),
                            perf_mode=DR)
                    if not zero_bias:
                        # pw2br host-scaled by WS
                        nc.tensor.matmul(ps2[:], cs["onesr"][:],
                                         cs["pw2br"][:],
                                         start=False, stop=True)
                    nc.vector.scalar_tensor_tensor(
                        out=X[:, t, :], in0=ps2[:], scalar=1.0 / WS,
                        in1=X[:, t, :], op0=ALU.mult, op1=ALU.add)

        def dump_dbg(i):
            if dbg:
                for t in range(NT):
                    Xc = spool.tile([P, D], f32, tag="dbgc", bufs=2,
                                    name="Xc")
                    nc.vector.tensor_copy(Xc[:], X[:, t, :])
                    nc.scalar.dma_start(
                        out=dbg[i].rearrange("(t p) d -> p t d", p=P)[:, t, :],
                        in_=Xc[:])

        # ---- pipeline -----------------------------------------------------
        if stages & 1:
            w1a = load_w("w1a", "w1", engs=(nc.gpsimd, nc.sync))
            w2a = load_w("w2a", "w2", engs=(nc.gpsimd, nc.sync))
            ff_module(w1a, cs["b1ca"], w2a, cs["b2ra"])
            dump_dbg(0)
        if stages & 2:
            mhsa_module()
            dump_dbg(1)
        if stages & 4:
            conv_module()
            dump_dbg(2)
        if stages & 8:
            w1b = load_w("w1b", "w1")
            w2b = load_w("w2b", "w2")
            ff_module(w1b, cs["b1cb"], w2b, cs["b2rb"])
            dump_dbg(3)

        # ---- final LN + store --------------------------------------------
        rstds, nmrs = emit_ln()
        stengs = [nc.sync, nc.gpsimd, nc.scalar]
        for t in range(NT):
            xo = spool.tile([P, D], f32, tag="xo", bufs=6, name="xo")
            nc.vector.tensor_scalar(
                out=xo[:], in0=X[:, t, :],
                scalar1=rstds[t // 4][:, t % 4:t % 4 + 1],
                scalar2=nmrs[t // 4][:, t % 4:t % 4 + 1],
                op0=ALU.mult, op1=ALU.add)
            if not trivial_final_gb:
                nc.vector.tensor_tensor(out=xo[:], in0=xo[:],
                                        in1=cs["lngr"][:], op=ALU.mult)
                nc.vector.tensor_tensor(out=xo[:], in0=xo[:],
                                        in1=cs["lnbr"][:], op=ALU.add)
            stengs[t % 3].dma_start(
                out=y_d.rearrange("(t p) d -> p t d", p=P)[:, t, :], in_=xo[:])


# ---------------------------------------------------------------------------
_cache = {}


def get_nc(debug_stages=False, trivial_final_gb=True, sim_safe=False,
           stages=15, zero_bias=False):
    key = ("nc", debug_stages, trivial_final_gb, sim_safe, stages, zero_bias)
    if key not in _cache:
        _install_hook()
        _cache[key] = build_nc(debug_stages, trivial_final_gb, sim_safe,
                               stages, zero_bias)
    return _cache[key]


def make_in_maps(inputs, debug_stages=False):
    prep = host_prep(inputs)
    x = np.asarray(inputs["inputs"], np.float32)
    trivial = (np.all(np.asarray(inputs["ln_g"]) == 1.0)
               and np.all(np.asarray(inputs["ln_b"]) == 0.0))
    zero_bias = all(
        not np.any(prep[k]) for k in ("b2ra", "b2rb", "bvr", "bor", "pw2br"))
    if not trivial:
        pass
    in_maps = []
    for c in range(NCORES):
        m = dict(prep)
        m["x"] = np.ascontiguousarray(x[c * BL:(c + 1) * BL].reshape(NTOK, D))
        in_maps.append(m)
    return in_maps, trivial, zero_bias


def kernel(**inputs):
    _install_hook()
    in_maps, trivial, zero_bias = make_in_maps(inputs)
    nc = get_nc(trivial_final_gb=trivial, zero_bias=zero_bias)
    res = run_bass_kernel_spmd(nc, in_maps, list(range(NCORES)))
    outs = [res.results[c]["y"].reshape(BL, L, D) for c in range(NCORES)]
    return np.concatenate(outs, axis=0)



# revision 34
# speedup vs baseline: 1.4159x; 1.0416x over previous
"""ChunkConformerBlock Trainium2 kernel.

Full inputs -> full output. Data-parallel over batch: B=16 sequences split
2-per-core across 8 NeuronCores; all parameters replicated. Each core runs
the whole conformer block (ff1 -> banded MHSA -> conv -> ff2 -> final LN)
on its 2048 tokens with activations kept in SBUF.

Layout strategy per core:
  - residual stream X: token-major fp32 tiles [128 tok, 512 d] (16 tiles)
  - LayerNorm: bn_stats/bn_aggr on DVE (token-major), fused scale+shift apply
  - matmul internals: feature-major bf16 (xn^T via PE transposes); weights
    pre-transposed host-side into [128, kchunks, N] bf16 SBUF layouts with
    LN gammas/betas, BN affine and 1/sqrt(dh) folded in
  - banded attention (band [i-6, i+2]): 136-wide key windows from guarded
    feature-major k^T; softmax token-major; attn transposed on PE for the
    attn@V matmul; shifted token-major v tiles so all matmul operands start
    at partition 0
  - depthwise causal conv K=32: 32 shifted fused multiply-adds
    (scalar_tensor_tensor) on DVE/GPSIMD over feature-major bf16 with
    zero guard columns per sequence
"""

import numpy as np
import ml_dtypes

import concourse.bass as bass
import concourse.tile as tile
from concourse import mybir
from concourse.bass_utils import run_bass_kernel_spmd

import json as _json

# ---------------------------------------------------------------------------
# walrus in this container supports at most ONE sync-wait command per
# instruction; Tile can emit more. Split extras onto standalone
# EventSemaphore instructions at the BIR-JSON level.
_wsplit_ctr = [0]


def _split_waits(bir_json: bytes, cap: int = 1) -> bytes:
    j = _json.loads(bir_json)
    changed = False
    for f in j.get("functions", []):
        for b in f.get("blocks", []):
            new_list = []
            for ins in b.get("instructions", []):
                si = ins.get("sync_info") or {}
                waits = si.get("on_wait") or []
                if len(waits) > cap:
                    extra, keep = waits[:-cap], waits[-cap:]
                    si["on_wait"] = keep
                    ins["sync_info"] = si
                    for w in extra:
                        _wsplit_ctr[0] += 1
                        new_list.append({
                            "engine": ins.get("engine"),
                            "ins": [], "outs": [],
                            "name": f"I-wsplit-{_wsplit_ctr[0]}",
                            "opcode": "EventSemaphore",
                            "sync_info": {"on_update": [], "on_wait": [w]},
                        })
                    changed = True
                new_list.append(ins)
            b["instructions"] = new_list
    return _json.dumps(j).encode() if changed else bir_json


_hook_installed = [False]


def _install_hook():
    if _hook_installed[0]:
        return
    from concourse import bass_utils as _bu, bass2jax as _b2j
    orig = _bu.compile_bir_kernel

    def patched(bir_json, tmpdir, neff_name="file.neff"):
        return orig(_split_waits(bir_json), tmpdir, neff_name)

    _bu.compile_bir_kernel = patched
    _b2j.compile_bir_kernel = patched
    _hook_installed[0] = True


# ---------------------------------------------------------------------------
f32 = mybir.dt.float32
bf16 = mybir.dt.bfloat16
f8 = mybir.dt.float8e4
AF = mybir.ActivationFunctionType
ALU = mybir.AluOpType
AX = mybir.AxisListType
DR = mybir.MatmulPerfMode.DoubleRow
bfnp = ml_dtypes.bfloat16
f8np = mybir.dt.np(mybir.dt.float8e4)
WS = 128.0                    # fp8 weight pre-scale (undone on psum read)
SCV = 64.0                    # dwconv output stored-scale (convT = 64*conv)
CS = SCV / (WS * WS)          # conv psum -> convT multiplier

NCORES = 8
B, L, D = 16, 1024, 512
BL = B // NCORES              # sequences per core
NTOK = BL * L                 # tokens per core
P = 128
NT = NTOK // P                # 16 token tiles
DC = D // P                   # 4 d-chunks
F = 4 * D                     # 2048 ff hidden
FCH = F // P                  # 16 f-chunks
H, DH = 4, 128
KC = 32                       # conv taps
WF, WB = 6, 2
WIN = 136                     # key window = 128 + WF + WB
FC = 0.5
EPS = 1e-3
GK = 8                        # kT/xnT guard columns each side
GC = 32                       # gluT guard columns (per sequence, front)
T4 = 512                      # wide token tile for N=512 matmuls
NT4 = NTOK // T4              # 4
SL = GC + L                   # per-seq glu row length
TPS = L // P                  # tiles per sequence (8)
DVE_TAPS = 32                 # dwconv taps on DVE (gpsimd lacks stt)


def _chunk_k(w, n_chunk):
    """[K, N] host weight -> [128, n_chunk, N] SBUF layout (K on partitions)."""
    K, N = w.shape
    assert K == n_chunk * P
    return np.ascontiguousarray(w.reshape(n_chunk, P, N).transpose(1, 0, 2))


def _bias_cols(b, n_chunk):
    """[N] bias -> [128, n_chunk] per-partition bias columns."""
    assert b.shape[0] == n_chunk * P
    return np.ascontiguousarray(b.reshape(n_chunk, P).T)


def _chunk_mask_np(n):
    i = np.arange(n)[:, None]
    j = np.arange(n)[None, :]
    low = np.maximum(i - WF, 0)
    high = np.clip(i + WB, 0, n)
    low = low - np.maximum(low - n + WB, 0)
    high = np.maximum(high, WB)
    return (j >= low) & (j <= high)


def host_prep(inputs):
    """Fold LN gammas/betas, BN affine, 1/sqrt(dh); build SBUF-layout arrays."""
    g = {k: np.asarray(v, np.float32) for k, v in inputs.items()}
    out = {}

    for pfx, tag in (("ff1", "a"), ("ff2", "b")):
        w1 = g[pfx + "_g"][:, None] * g[pfx + "_w1"]
        b1 = g[pfx + "_b"] @ g[pfx + "_w1"] + g[pfx + "_b1"]
        out["w1" + tag] = _chunk_k(w1 * WS, DC).astype(f8np)
        out["b1c" + tag] = _bias_cols(b1, FCH)
        out["w2" + tag] = _chunk_k(g[pfx + "_w2"] * WS, FCH).astype(f8np)
        out["b2r" + tag] = (g[pfx + "_b2"] * WS)[None, :].astype(bfnp)

    sc = 1.0 / np.sqrt(DH)
    for nm, scale in (("wq", sc), ("wk", 1.0), ("wv", 1.0)):
        wf = g[nm].reshape(D, H * DH)
        bf_ = g["b" + nm[1]].reshape(H * DH)
        wp = (g["mh_g"][:, None] * wf) * scale
        bp = (g["mh_b"] @ wf + bf_) * scale
        out[nm + "s"] = _chunk_k(wp * WS, DC).astype(f8np)
        if nm != "wv":
            out["b" + nm[1] + "c"] = _bias_cols(bp, H)
        else:
            out["bvr"] = (bp * WS)[None, :].astype(bfnp)
    out["wos"] = _chunk_k(g["wo"].reshape(H * DH, D) * WS, H).astype(f8np)
    out["bor"] = (g["bo"] * WS)[None, :].astype(bfnp)

    pw1 = g["cv_g"][:, None] * g["pw1_w"]
    pw1b = g["cv_b"] @ g["pw1_w"] + g["pw1_b"]
    out["pw1s"] = _chunk_k(pw1 * WS, DC).astype(f8np)
    # a-half bias pre-scaled by WS so gluT = WS * glu_true (one stt keeps
    # (psa + WS*b_a) * sigmoid); gate-half bias unscaled (applied after
    # scale=1/WS inside the Sigmoid activation).
    pw1b_sc = pw1b.copy()
    pw1b_sc[:D] *= WS
    out["pw1bc"] = _bias_cols(pw1b_sc, 2 * DC)
    s = g["bn_g"] / np.sqrt(g["bn_v"] + EPS)
    t = g["bn_b"] - g["bn_m"] * s
    sepw = g["sep_w"] * s[None, :]
    sepb = g["sep_b"] * s + t
    out["seps"] = _chunk_k(sepw * WS, DC).astype(f8np)
    out["sepbc"] = _bias_cols(sepb, 2 * DC)
    out["pw2s"] = _chunk_k(g["pw2_w"] * WS, 2 * DC).astype(f8np)
    out["pw2br"] = (g["pw2_b"] * WS)[None, :].astype(bfnp)

    out["dww"] = np.ascontiguousarray(
        g["dw_w"].T.reshape(DC, P, KC).transpose(1, 0, 2)).astype(np.float32)
    # paired diag taps for fp8 DoubleRow dwconv:
    # dgall[p, dcc, kp, i, m] = WS * dw_w[kp + 16*i, dcc*128+p] iff m == p
    dg = np.zeros((P, DC, KC // 2, 2, P), np.float32)
    for dcc in range(DC):
        for kp in range(KC // 2):
            for i in range(2):
                w = g["dw_w"][kp + 16 * i, dcc * P:(dcc + 1) * P] * WS
                dg[np.arange(P), dcc, kp, i, np.arange(P)] = w
    out["dgall"] = dg.astype(f8np)

    out["lngr"] = np.broadcast_to(g["ln_g"][None, :], (P, D)).astype(np.float32).copy()
    out["lnbr"] = np.broadcast_to(g["ln_b"][None, :], (P, D)).astype(np.float32).copy()

    m_full = _chunk_mask_np(L)
    masks = np.full((P, 3, WIN), -1e9, np.float32)
    for mi, t in ((0, 0), (1, 3), (2, TPS - 1)):
        i0 = t * P
        for r in range(P):
            for c in range(WIN):
                jj = i0 - WF + c
                if 0 <= jj < L and m_full[i0 + r, jj]:
                    masks[r, mi, c] = 0.0
    out["masks"] = masks
    out["masks2"] = np.repeat(masks[:, :, None, :], 2, axis=2).astype(bfnp)

    out["ident"] = np.eye(P, dtype=np.float32).astype(bfnp)
    out["onesr"] = np.ones((1, P), np.float32).astype(bfnp)
    out["b2ra32"] = out["b2ra"].astype(np.float32)
    out["b2rb32"] = out["b2rb"].astype(np.float32)
    out["bor32"] = out["bor"].astype(np.float32)
    out["pw2br32"] = out["pw2br"].astype(np.float32)
    out["bvr32"] = out["bvr"].astype(np.float32)
    return out


SPECS = {
    "w1a": ([P, DC, F], f8), "b1ca": ([P, FCH], f32),
    "w2a": ([P, FCH, D], f8), "b2ra": ([1, D], bf16),
    "w1b": ([P, DC, F], f8), "b1cb": ([P, FCH], f32),
    "w2b": ([P, FCH, D], f8), "b2rb": ([1, D], bf16),
    "wqs": ([P, DC, H * DH], f8), "bqc": ([P, H], f32),
    "wks": ([P, DC, H * DH], f8), "bkc": ([P, H], f32),
    "wvs": ([P, DC, H * DH], f8), "bvr": ([1, H * DH], bf16),
    "wos": ([P, H, D], f8), "bor": ([1, D], bf16),
    "pw1s": ([P, DC, 2 * D], f8), "pw1bc": ([P, 2 * DC], f32),
    "seps": ([P, DC, 2 * D], f8), "sepbc": ([P, 2 * DC], f32),
    "pw2s": ([P, 2 * DC, D], f8), "pw2br": ([1, D], bf16),
    "dww": ([P, DC, KC], f32),
    "dgall": ([P, DC, KC // 2, 2, P], f8),
    "lngr": ([P, D], f32), "lnbr": ([P, D], f32),
    "masks": ([P, 3, WIN], f32),
    "masks2": ([P, 3, 2, WIN], bf16),
    "ident": ([P, P], bf16), "onesr": ([1, P], bf16),
    "b2ra32": ([1, D], f32), "b2rb32": ([1, D], f32),
    "bor32": ([1, D], f32), "pw2br32": ([1, D], f32),
    "bvr32": ([1, H * DH], f32),
}


# ---------------------------------------------------------------------------
def build_nc(debug_stages=False, trivial_final_gb=True, sim_safe=False, stages=15, zero_bias=False):
    nc = bass.Bass()

    x_d = nc.dram_tensor("x", [NTOK, D], f32, kind="ExternalInput")
    y_d = nc.dram_tensor("y", [NTOK, D], f32, kind="ExternalOutput")
    dbg = []
    if debug_stages:
        for i in range(4):
            dbg.append(nc.dram_tensor(f"dbg{i}", [NTOK, D], f32,
                                      kind="ExternalOutput"))

    wd = {}
    for nm, (shp, dt) in SPECS.items():
        wd[nm] = nc.dram_tensor(nm, shp, dt, kind="ExternalInput")

    with tile.TileContext(nc) as tc:
        _emit(nc, tc, x_d, y_d, wd, dbg, trivial_final_gb, sim_safe, stages, zero_bias)
    return nc


def _emit(nc, tc, x_d, y_d, wd, dbg, trivial_final_gb, sim_safe, stages=15, zero_bias=False):
    from contextlib import ExitStack
    ctx = ExitStack()
    with ctx:
        cpool = ctx.enter_context(tc.tile_pool(name="const", bufs=1))
        wpool = ctx.enter_context(tc.tile_pool(name="wts", bufs=1))
        xpool = ctx.enter_context(tc.tile_pool(name="xres", bufs=1))
        apool = ctx.enter_context(tc.tile_pool(name="acts", bufs=1))
        bpool = ctx.enter_context(tc.tile_pool(name="big", bufs=3))
        spool = ctx.enter_context(tc.tile_pool(name="small", bufs=2))
        stpool = ctx.enter_context(tc.tile_pool(name="stats", bufs=2))
        ppool = ctx.enter_context(tc.tile_pool(name="ps", bufs=3, space="PSUM"))
        pspool = ctx.enter_context(tc.tile_pool(name="pss", bufs=1, space="PSUM"))


        def load_w(nm, tag, engs=(nc.gpsimd,), grp=None):
            shp, dt = SPECS[nm]
            t = wpool.tile(shp, dt, tag=tag, name=nm + "_sb")
            if len(shp) == 3 and shp[1] > 1:
                g = grp or 1
                for i, c in enumerate(range(0, shp[1], g)):
                    engs[i % len(engs)].dma_start(
                        out=t[:, c:c + g, :], in_=wd[nm][:, c:c + g, :])
            else:
                engs[0].dma_start(out=t[:], in_=wd[nm][:])
            return t

        # ---- startup: ff1 weights first on gpsimd, ident first on scalar,
        # X round-robin across the three DMA-capable queues ----------------
        cs = {}

        def load_c(nm, eng):
            shp, dt = SPECS[nm]
            t = cpool.tile(shp, dt, tag=nm, name=nm)
            eng.dma_start(out=t[:], in_=wd[nm][:])
            cs[nm] = t

        w1a = w2a = None
        if stages & 1:
            w1a = load_w("w1a", "w1", engs=(nc.gpsimd,), grp=2)
            w2a = load_w("w2a", "w2", engs=(nc.gpsimd,), grp=8)
        load_c("ident", nc.scalar)

        dmaengs = [nc.sync, nc.scalar, nc.gpsimd]
        X = xpool.tile([P, NT, D], f32, tag="X", name="X")
        xr = x_d.rearrange("(t p) d -> p t d", p=P)
        for t in range(NT):
            dmaengs[t % 3].dma_start(out=X[:, t, :], in_=xr[:, t, :])

        cnames = ["b1ca", "b2ra", "b1cb", "b2rb", "bqc", "bkc", "bvr",
                  "bor", "pw1bc", "sepbc", "pw2br",
                  "masks", "onesr"]
        if not trivial_final_gb:
            cnames += ["lngr", "lnbr"]
        for nm in cnames:
            load_c(nm, nc.scalar)
        eps_t = cpool.tile([P, 1], f32, tag="eps", name="eps_t")
        nc.vector.memset(eps_t[:], EPS)

        # ---- helpers ------------------------------------------------------
        def bias_accum(ps_ap, nm, width, rows=P):
            nc.gpsimd.dma_start(
                out=ps_ap, in_=wd[nm][:].to_broadcast([rows, width]),
                accum_op=ALU.add)

        def emit_silu(out_ap, psum_ap, bias_ap, scale=1.0):
            nc.scalar.activation(out=out_ap, in_=psum_ap, func=AF.Silu,
                                 bias=bias_ap, scale=scale)

        def emit_ln():
            rstds, nmrs = [], []
            for g in range(NT // 4):
                mv = stpool.tile([P, 4, 2], f32, tag="mv", bufs=3, name="mv")
                for j in range(4):
                    st6 = stpool.tile([P, 6], f32, tag="st6", bufs=3,
                                      name="st6")
                    nc.vector.bn_stats(out=st6[:], in_=X[:, g * 4 + j, :])
                    nc.vector.bn_aggr(out=mv[:, j, :], in_=st6[:])
                rstd = stpool.tile([P, 4], f32, tag="rstd", bufs=3,
                                   name="rstd")
                nmr = stpool.tile([P, 4], f32, tag="nmr", bufs=3, name="nmr")
                nc.scalar.activation(out=rstd[:], in_=mv[:, :, 1],
                                     func=AF.Sqrt, bias=eps_t[:])
                nc.vector.reciprocal(out=rstd[:], in_=rstd[:])
                nc.vector.scalar_tensor_tensor(
                    out=nmr[:], in0=mv[:, :, 0], scalar=-1.0, in1=rstd[:],
                    op0=ALU.mult, op1=ALU.mult)
                rstds.append(rstd)
                nmrs.append(nmr)
            return rstds, nmrs

        def emit_xnT(rstds, nmrs, guard=False, dt=bf16, tag="xnT",
                     ceng=None):
            xnT = apool.tile([P, DC, NTOK + 2 * GK], dt, tag=tag,
                             name=tag)
            if guard:
                for dcc in range(DC):
                    nc.gpsimd.memset(xnT[:, dcc, 0:GK], 0.0)
                    nc.gpsimd.memset(xnT[:, dcc, GK + NTOK:], 0.0)
            for t in range(NT):
                xn = spool.tile([P, D], bf16, tag="xn", bufs=3, name="xn")
                nc.vector.tensor_scalar(
                    out=xn[:], in0=X[:, t, :],
                    scalar1=rstds[t // 4][:, t % 4:t % 4 + 1],
                    scalar2=nmrs[t // 4][:, t % 4:t % 4 + 1],
                    op0=ALU.mult, op1=ALU.add)
                pt = pspool.tile([P, D], bf16, tag="ptr", bufs=1, name="pt")
                for dcc in range(DC):
                    nc.tensor.transpose(pt[:, dcc * P:(dcc + 1) * P],
                                        xn[:, dcc * P:(dcc + 1) * P],
                                        cs["ident"][:])
                if ceng is nc.vector:
                    nc.vector.tensor_copy(
                        xnT[:, :, GK + t * P: GK + (t + 1) * P],
                        pt[:].rearrange("p (c t) -> p c t", c=DC))
                else:
                    nc.scalar.activation(
                        out=xnT[:, :, GK + t * P: GK + (t + 1) * P],
                        in_=pt[:].rearrange("p (c t) -> p c t", c=DC),
                        func=AF.Copy)
            return xnT

        def ff_module(w1, b1c, w2, b2r):
            rstds, nmrs = emit_ln()
            xnT = emit_xnT(rstds, nmrs, dt=f8, tag="xnT8", ceng=nc.vector)
            for t4 in range(NT4):
                tok0 = t4 * T4
                h1T = bpool.tile([P, FCH, T4], f8, tag="big", name="h1T")
                for fc in range(FCH):
                    ps = ppool.tile([P, T4], f32, tag="pbig", name="psf")
                    for c in range(DC // 2):
                        nc.tensor.matmul(
                            ps[:], w1[:, 2 * c:2 * c + 2, fc * P:(fc + 1) * P],
                            xnT[:, 2 * c:2 * c + 2, GK + tok0: GK + tok0 + T4],
                            start=(c == 0), stop=(c == DC // 2 - 1),
                            perf_mode=DR)
                    emit_silu(h1T[:, fc, :], ps[:], b1c[:, fc:fc + 1],
                              scale=1.0 / WS)
                for j in range(T4 // P):
                    t = (tok0 // P) + j
                    ps2 = ppool.tile([P, D], f32, tag="pbig", name="psb")
                    for c in range(FCH // 2):
                        nc.tensor.matmul(
                            ps2[:], h1T[:, 2 * c:2 * c + 2, j * P:(j + 1) * P],
                            w2[:, 2 * c:2 * c + 2, :],
                            start=(c == 0),
                            stop=(zero_bias and c == FCH // 2 - 1),
                            perf_mode=DR)
                    if not zero_bias:
                        # b2r host-scaled by WS so the FC/WS unscale is right
                        nc.tensor.matmul(ps2[:], cs["onesr"][:], b2r[:],
                                         start=False, stop=True)
                    nc.vector.scalar_tensor_tensor(
                        out=X[:, t, :], in0=ps2[:], scalar=FC / WS,
                        in1=X[:, t, :], op0=ALU.mult, op1=ALU.add)

        def mhsa_module():
            rstds, nmrs = emit_ln()
            xnT = emit_xnT(rstds, nmrs, guard=True, dt=f8, tag="xnT8")
            wq = load_w("wqs", "wq", grp=2)
            wk = load_w("wks", "wk", grp=2)
            wv = load_w("wvs", "wv", grp=2)
            wo = load_w("wos", "wo", grp=2)

            qT = bpool.tile([P, H, NTOK], bf16, tag="big", name="qT")
            kT = bpool.tile([P, H, NTOK + 2 * GK], bf16, tag="big", name="kT")
            for h in range(H):
                nc.gpsimd.memset(kT[:, h, 0:GK], 0.0)
                nc.gpsimd.memset(kT[:, h, GK + NTOK:], 0.0)
            for h in range(H):
                for t4 in range(NT4):
                    tok0 = t4 * T4
                    psq = ppool.tile([P, T4], f32, tag="pbig", name="psq")
                    for c in range(DC // 2):
                        nc.tensor.matmul(
                            psq[:], wq[:, 2 * c:2 * c + 2,
                                       h * DH:(h + 1) * DH],
                            xnT[:, 2 * c:2 * c + 2, GK + tok0: GK + tok0 + T4],
                            start=(c == 0), stop=(c == DC // 2 - 1),
                            perf_mode=DR)
                    nc.vector.tensor_scalar(
                        out=qT[:, h, tok0:tok0 + T4], in0=psq[:],
                        scalar1=1.0 / WS, scalar2=cs["bqc"][:, h:h + 1],
                        op0=ALU.mult, op1=ALU.add)
                    psk = ppool.tile([P, T4], f32, tag="pbig", name="psk")
                    for c in range(DC // 2):
                        nc.tensor.matmul(
                            psk[:], wk[:, 2 * c:2 * c + 2,
                                       h * DH:(h + 1) * DH],
                            xnT[:, 2 * c:2 * c + 2, GK + tok0: GK + tok0 + T4],
                            start=(c == 0), stop=(c == DC // 2 - 1),
                            perf_mode=DR)
                    nc.vector.tensor_scalar(
                        out=kT[:, h, GK + tok0: GK + tok0 + T4], in0=psk[:],
                        scalar1=1.0 / WS, scalar2=cs["bkc"][:, h:h + 1],
                        op0=ALU.mult, op1=ALU.add)

            # shifted v tiles: tile st covers tokens [st*128-6, st*128+122)
            v_sb = bpool.tile([P, NT, H * DH], bf16, tag="big", name="v_sb")
            v17 = spool.tile([P, H * DH], bf16, tag="v17", bufs=1, name="v17")
            for st in range(NT + 1):
                m = P if st < NT else GK
                ps = ppool.tile([P, H * DH], f32, tag="pbig", name="psv")
                c0 = GK + st * P - WF
                for c in range(DC // 2):
                    lhsT = xnT[:, 2 * c:2 * c + 2, c0:c0 + m]
                    nc.tensor.matmul(
                        ps[:m, :], lhsT, wv[:, 2 * c:2 * c + 2, :],
                        start=(c == 0),
                        stop=(zero_bias and c == DC // 2 - 1),
                        perf_mode=DR)
                if not zero_bias:
                    nc.tensor.matmul(ps[:m, :], cs["onesr"][:, 0:m],
                                     cs["bvr"][:], start=False, stop=True)
                dst = v_sb[:, st, :] if st < NT else v17[0:GK, :]
                nc.vector.tensor_scalar(dst, in0=ps[:m, :],
                                        scalar1=1.0 / WS, scalar2=None,
                                        op0=ALU.mult)

            for st in range(NT):
                i0 = st * P
                tin = st % TPS
                mi = 0 if tin == 0 else (2 if tin == TPS - 1 else 1)
                oT_t = spool.tile([P, H, P], f8, tag="oTt", bufs=2,
                                  name="oT_t")
                for hp in range(H // 2):
                    sps = pspool.tile([P, 2, WIN], f32, tag="psc", bufs=1,
                                      name="sps")
                    for hh in range(2):
                        h = hp * 2 + hh
                        nc.tensor.matmul(
                            sps[:, hh, :], qT[:, h, i0:i0 + P],
                            kT[:, h, GK + i0 - WF: GK + i0 - WF + WIN],
                            start=True, stop=True)
                    sm = spool.tile([P, 2, WIN], f32, tag="sm", name="sm")
                    for hh in range(2):
                        nc.vector.tensor_tensor(
                            out=sm[:, hh, :], in0=sps[:, hh, :],
                            in1=cs["masks"][:, mi, :], op=ALU.add)
                    for hh in range(2):
                        h = hp * 2 + hh
                        ex = spool.tile([P, WIN], bf16, tag="ex", name="ex")
                        den = spool.tile([P, 1], f32, tag="den", name="den")
                        nc.scalar.activation(
                            out=ex[:], in_=sm[:, hh, :], func=AF.Exp,
                            bias=0.0, accum_out=den[:])
                        rden = spool.tile([P, 1], f32, tag="rden", name="rden")
                        nc.vector.reciprocal(out=rden[:], in_=den[:])
                        at = spool.tile([P, WIN], bf16, tag="at", name="at")
                        nc.vector.tensor_scalar_mul(at[:], ex[:], rden[:])
                        pt2 = pspool.tile([P, 2 * P], bf16, tag="pt2", bufs=2,
                                          name="pt2")
                        nc.tensor.transpose(pt2[:, 0:P], at[:, 0:P],
                                            cs["ident"][:])
                        nc.tensor.transpose(pt2[0:GK, P:P + P], at[:, P:WIN],
                                            cs["ident"][:])
                        aT = spool.tile([P, P], bf16, tag="aT", name="aT")
                        bT = spool.tile([GK, P], bf16, tag="bT", name="bT")
                        nc.scalar.activation(out=aT[:], in_=pt2[:, 0:P],
                                             func=AF.Copy)
                        nc.vector.tensor_copy(bT[:], pt2[0:GK, P:P + P])
                        po = pspool.tile([P, P], f32, tag="pav", bufs=1,
                                         name="po")
                        nc.tensor.matmul(po[:],
                                         v_sb[:, st, h * DH:(h + 1) * DH],
                                         aT[:], start=True, stop=False)
                        vn = (v_sb[0:GK, st + 1, h * DH:(h + 1) * DH]
                              if st + 1 < NT else v17[0:GK, :][:, h * DH:(h + 1) * DH])
                        nc.tensor.matmul(po[:, P - GK:P], vn,
                                         bT[:, P - GK:P],
                                         start=False, stop=True)
                        nc.scalar.activation(out=oT_t[:, h, :], in_=po[:],
                                             func=AF.Copy)
                # output projection + residual for this tile
                pso = ppool.tile([P, D], f32, tag="pbig", name="pso")
                for i in range(H // 2):
                    nc.tensor.matmul(pso[:], oT_t[:, 2 * i:2 * i + 2, :],
                                     wo[:, 2 * i:2 * i + 2, :],
                                     start=(i == 0),
                                     stop=(zero_bias and i == H // 2 - 1),
                                     perf_mode=DR)
                if not zero_bias:
                    nc.tensor.matmul(pso[:], cs["onesr"][:], cs["bor"][:],
                                     start=False, stop=True)
                nc.vector.scalar_tensor_tensor(
                    out=X[:, st, :], in0=pso[:], scalar=1.0 / WS,
                    in1=X[:, st, :], op0=ALU.mult, op1=ALU.add)

        def conv_module():
            rstds, nmrs = emit_ln()
            xnT = emit_xnT(rstds, nmrs, dt=f8, tag="xnT8", ceng=nc.vector)
            pw1 = load_w("pw1s", "pw1", grp=2)
            dgall = load_w("dgall", "dgall")
            gluT = bpool.tile([P, DC, 2, SL], f8, tag="big", name="gluT")
            for dcc in range(DC):
                for s in range(2):
                    nc.gpsimd.memset(gluT[:, dcc, s, 0:GC], 0.0)
            for fc in range(DC):
                for t4 in range(NT4):
                    tok0 = t4 * T4
                    s, hf = t4 // 2, t4 % 2
                    psa = ppool.tile([P, T4], f32, tag="pbig", name="psa")
                    psg = ppool.tile([P, T4], f32, tag="pbig", name="psg")
                    for c in range(DC // 2):
                        nc.tensor.matmul(
                            psa[:], pw1[:, 2 * c:2 * c + 2,
                                        fc * P:(fc + 1) * P],
                            xnT[:, 2 * c:2 * c + 2, GK + tok0: GK + tok0 + T4],
                            start=(c == 0), stop=(c == DC // 2 - 1),
                            perf_mode=DR)
                    for c in range(DC // 2):
                        nc.tensor.matmul(
                            psg[:], pw1[:, 2 * c:2 * c + 2,
                                        D + fc * P: D + (fc + 1) * P],
                            xnT[:, 2 * c:2 * c + 2, GK + tok0: GK + tok0 + T4],
                            start=(c == 0), stop=(c == DC // 2 - 1),
                            perf_mode=DR)
                    sg = spool.tile([P, T4], bf16, tag="sg", name="sg")
                    nc.scalar.activation(out=sg[:], in_=psg[:], func=AF.Sigmoid,
                                         bias=cs["pw1bc"][:, DC + fc:DC + fc + 1],
                                         scale=1.0 / WS)
                    # gluT = (psa + WS*b_a) * sg = WS * glu_true  (fp8)
                    nc.vector.scalar_tensor_tensor(
                        out=gluT[:, fc, s, GC + hf * T4: GC + (hf + 1) * T4],
                        in0=psa[:], scalar=cs["pw1bc"][:, fc:fc + 1], in1=sg[:],
                        op0=ALU.add, op1=ALU.mult)

            sep = load_w("seps", "sep", grp=2)
            convT = bpool.tile([P, DC, 2, L], f8, tag="big", name="convT")
            # depthwise conv: 16 paired-tap fp8 DoubleRow diagonal matmuls
            # per (chunk, 512-token tile); rhs pairs taps (k, k+16) via a
            # stride-16 middle AP dim over the shifted glu slice.
            for t4 in range(NT4):
                for dcc in range(DC):
                    s, hf = t4 // 2, t4 % 2
                    psc_ = ppool.tile([P, T4], f32, tag="pbig", name="psc_")
                    for kp in range(KC // 2):
                        rhs = gluT[:, dcc, s,
                                   1 + hf * T4 + kp: 1 + hf * T4 + kp + T4]
                        rhs = rhs.unsqueeze(1)
                        rhs.ap[1] = (16, 2)
                        nc.tensor.matmul(
                            psc_[:], dgall[:, dcc, kp, :, :], rhs,
                            start=(kp == 0), stop=(kp == KC // 2 - 1),
                            perf_mode=DR)
                    # convT = SCV * conv_true (fp8)
                    nc.vector.tensor_scalar(
                        out=convT[:, dcc, s, hf * T4:(hf + 1) * T4],
                        in0=psc_[:], scalar1=CS, scalar2=None, op0=ALU.mult)

            pw2 = load_w("pw2s", "pw2", grp=4)
            for t4 in range(NT4):
                tok0 = t4 * T4
                s, hf = t4 // 2, t4 % 2
                silT = bpool.tile([P, 2 * DC, T4], f8, tag="big",
                                  name="silT")
                for fc in range(2 * DC):
                    ps = ppool.tile([P, T4], f32, tag="pbig", name="pss1")
                    for c in range(DC // 2):
                        nc.tensor.matmul(
                            ps[:], sep[:, 2 * c:2 * c + 2,
                                       fc * P:(fc + 1) * P],
                            convT[:, 2 * c:2 * c + 2, s,
                                  hf * T4:(hf + 1) * T4],
                            start=(c == 0), stop=(c == DC // 2 - 1),
                            perf_mode=DR)
                    emit_silu(silT[:, fc, :], ps[:], cs["sepbc"][:, fc:fc + 1],
                              scale=1.0 / (SCV * WS))
                for j in range(T4 // P):
                    t = (tok0 // P) + j
                    ps2 = ppool.tile([P, D], f32, tag="pbig", name="pss2")
                    for c in range(DC):
                        nc.tensor.matmul(
                            ps2[:], silT[:, 2 * c:2 * c + 2,
                                         j * P:(j + 1) * P],
                            pw2[:, 2 * c:2 * c + 2, :], start=(c == 0),
                            stop=(zero_bias and c == DC - 1),
                            perf_mode=DR)
                    if not zero_bias:
                        # pw2br host-scaled by WS
                        nc.tensor.matmul(ps2[:], cs["onesr"][:],
                                         cs["pw2br"][:],
                                         start=False, stop=True)
                    nc.vector.scalar_tensor_tensor(
                        out=X[:, t, :], in0=ps2[:], scalar=1.0 / WS,
                        in1=X[:, t, :], op0=ALU.mult, op1=ALU.add)

        def dump_dbg(i):
            if dbg:
                for t in range(NT):
                    Xc = spool.tile([P, D], f32, tag="dbgc", bufs=2,
                                    name="Xc")
                    nc.vector.tensor_copy(Xc[:], X[:, t, :])
                    nc.scalar.dma_start(
                        out=dbg[i].rearrange("(t p) d -> p t d", p=P)[:, t, :],
                        in_=Xc[:])

        # ---- pipeline -----------------------------------------------------
        if stages & 1:
            ff_module(w1a, cs["b1ca"], w2a, cs["b2ra"])
            dump_dbg(0)
        if stages & 2:
            mhsa_module()
            dump_dbg(1)
        if stages & 4:
            conv_module()
            dump_dbg(2)
        if stages & 8:
            w1b = load_w("w1b", "w1", grp=2)
            w2b = load_w("w2b", "w2", grp=8)
            ff_module(w1b, cs["b1cb"], w2b, cs["b2rb"])
            dump_dbg(3)

        # ---- final LN + store --------------------------------------------
        rstds, nmrs = emit_ln()
        stengs = [nc.sync, nc.gpsimd, nc.scalar]
        for t in range(NT):
            xo = spool.tile([P, D], f32, tag="xo", bufs=6, name="xo")
            xeng = nc.vector if t % 2 == 0 else nc.gpsimd
            xeng.tensor_scalar(
                out=xo[:], in0=X[:, t, :],
                scalar1=rstds[t // 4][:, t % 4:t % 4 + 1],
                scalar2=nmrs[t // 4][:, t % 4:t % 4 + 1],
                op0=ALU.mult, op1=ALU.add)
            if not trivial_final_gb:
                nc.vector.tensor_tensor(out=xo[:], in0=xo[:],
                                        in1=cs["lngr"][:], op=ALU.mult)
                nc.vector.tensor_tensor(out=xo[:], in0=xo[:],
                                        in1=cs["lnbr"][:], op=ALU.add)
            stengs[t % 3].dma_start(
                out=y_d.rearrange("(t p) d -> p t d", p=P)[:, t, :], in_=xo[:])


# ---------------------------------------------------------------------------
_cache = {}


def get_nc(debug_stages=False, trivial_final_gb=True, sim_safe=False,
           stages=15, zero_bias=False):
    key = ("nc", debug_stages, trivial_final_gb, sim_safe, stages, zero_bias)
    if key not in _cache:
        _install_hook()
        _cache[key] = build_nc(debug_stages, trivial_final_gb, sim_safe,
                               stages, zero_bias)
    return _cache[key]


def make_in_maps(inputs, debug_stages=False):
    prep = host_prep(inputs)
    x = np.asarray(inputs["inputs"], np.float32)
    trivial = (np.all(np.asarray(inputs["ln_g"]) == 1.0)
               and np.all(np.asarray(inputs["ln_b"]) == 0.0))
    zero_bias = all(
        not np.any(prep[k]) for k in ("b2ra", "b2rb", "bvr", "bor", "pw2br"))
    if not trivial:
        pass
    in_maps = []
    for c in range(NCORES):
        m = dict(prep)
        m["x"] = np.ascontiguousarray(x[c * BL:(c + 1) * BL].reshape(NTOK, D))
        in_maps.append(m)
    return in_maps, trivial, zero_bias


def kernel(**inputs):
    _install_hook()
    in_maps, trivial, zero_bias = make_in_maps(inputs)
    nc = get_nc(trivial_final_gb=trivial, zero_bias=zero_bias)
    res = run_bass_kernel_spmd(nc, in_maps, list(range(NCORES)))
    outs = [res.results[c]["y"].reshape(BL, L, D) for c in range(NCORES)]
    return np.concatenate(outs, axis=0)

